# revision 1
# baseline (speedup 1.0000x reference)
"""RWKV-v4 block (time-mix WKV attention + channel-mix GLU) on 8 Trainium2
NeuronCores, data-parallel over batch B.

Layouts per core (B_local=4, T=1024, C=512, H=2048):
  - layout A: [t(128p), n(8), c(512)]  -- LayerNorm (per-partition stats),
    residual adds, final store.
  - layout B: [c(128p) x 4 chunks, t(1024)] -- mixing, WKV scan (along free
    dim), k/v/r matmuls.
  - A->B via bf16 DMA transpose through a DRAM bounce; B->A avoided by
    running Wo/cWv/cWr matmuls with the *activation* as the stationary
    operand (out = act.T @ W.T lands in layout A).

WKV: with per-channel M = max_t k, e=exp(k-M), the recurrence
  P_t = d*P_{t-1} + e_t*v_t,  Q_t = d*Q_{t-1} + e_t   (d = exp(-exp(decay)))
  y_t = (P_{t-1} + exp(u)*e_t*v_t) / (Q_{t-1} + exp(u)*e_t)
matches the reference's max-tracking scan exactly (the M scaling cancels in
the ratio).  Runs as two native tensor_tensor_scan ops per 128-channel chunk.
"""

import numpy as np
import ml_dtypes
from contextlib import ExitStack

import concourse.bass as bass
import concourse.tile as tile
from concourse import bacc, mybir

B, T, C = 32, 1024, 512
H = 4 * C
NCORES = 8
BL = B // NCORES  # batches per core
NT = T // 128     # 8 t-subtiles per batch
CC = C // 128     # 4 channel chunks
HC = H // 128     # 16 hidden chunks

F32 = mybir.dt.float32
BF16 = mybir.dt.bfloat16
AX = mybir.AxisListType
OP = mybir.AluOpType
AF = mybir.ActivationFunctionType


PHASE_LOG = []


def _emit(nc, tc, ctx, io, bl):
    """Emit the whole per-core program."""
    PHASE_LOG.clear()

    def mark(label):
        PHASE_LOG.append((nc.next_id(), label))

    x_d = io["x"].ap()
    y_d = io["y"].ap()

    def col(name, c0):  # [128,1] slice of a [N] dram vector
        return io[name].ap()[c0 * 128:(c0 + 1) * 128].rearrange(
            "(c one) -> c one", one=1)

    sb = ctx.enter_context(tc.tile_pool(name="sb", bufs=1))
    ps = ctx.enter_context(tc.tile_pool(name="ps", bufs=8, space="PSUM"))
    dramp = ctx.enter_context(tc.tile_pool(name="dram", bufs=2, space="DRAM"))

    # ---- constants / weights (resident) ----
    def load_w(name, rows, cols):
        tiles = []
        for i in range(rows // 128):
            t_ = sb.tile([128, cols], BF16, tag=f"w_{name}_{i}")
            nc.sync.dma_start(t_[:], io[name].ap()[i * 128:(i + 1) * 128, :])
            tiles.append(t_)
        return tiles

    wkT = load_w("wkT", C, C)
    wvT = load_w("wvT", C, C)
    wrT = load_w("wrT", C, C)
    woT = load_w("woT", C, C)
    cwkT = load_w("cwkT", C, H)
    cwvT = load_w("cwvT", H, C)
    cwrT = load_w("cwrT", C, C)

    def vec4(name):
        ts_ = []
        for i in range(CC):
            t_ = sb.tile([128, 1], F32, tag=f"v_{name}_{i}")
            nc.sync.dma_start(t_[:], col(name, i))
            ts_.append(t_)
        return ts_

    delta_c = vec4("delta")
    expu_c = vec4("expu")
    vb_c = vec4("vb")
    rb_c = vec4("rb")

    def vec4_m1(name):
        # coef - 1 (mix runs as o = xn + (coef-1)*d so every operand reads
        # at an aligned offset; the t-shift lives only inside d)
        ts_ = []
        for i in range(CC):
            t_ = sb.tile([128, 1], F32, tag=f"vm_{name}_{i}")
            nc.sync.dma_start(t_[:], col(name, i))
            nc.vector.tensor_scalar_add(t_[:], t_[:], -1.0)
            ts_.append(t_)
        return ts_

    tmk_c = vec4_m1("tmk")
    tmv_c = vec4_m1("tmv")
    tmr_c = vec4_m1("tmr")
    cmk_c = vec4_m1("cmk")
    cmr_c = vec4_m1("cmr")
    kkb_c = []
    for i in range(HC):
        t_ = sb.tile([128, 1], F32, tag=f"v_kkb_{i}")
        nc.sync.dma_start(t_[:], col("kkb", i))
        kkb_c.append(t_)

    eps_t = sb.tile([128, 1], F32, tag="eps")
    nc.vector.memset(eps_t[:], 1e-5)
    zrow = sb.tile([32, C], BF16, tag="zrow")
    nc.vector.memset(zrow[:], 0.0)


    # ---- per-batch pipeline ----
    xa_pool = ctx.enter_context(tc.tile_pool(name="xa", bufs=1))
    x1_pool = ctx.enter_context(tc.tile_pool(name="x1", bufs=1))
    lnp = ctx.enter_context(tc.tile_pool(name="ln", bufs=1))
    bp = ctx.enter_context(tc.tile_pool(name="bp", bufs=1))
    wkvp = ctx.enter_context(tc.tile_pool(name="wkv", bufs=1))
    srwp = ctx.enter_context(tc.tile_pool(name="srw", bufs=1))
    cmp_ = ctx.enter_context(tc.tile_pool(name="cm", bufs=1))
    outp = ctx.enter_context(tc.tile_pool(name="out", bufs=2))

    def layer_norm(src_tile, which):
        """src [128, NT, 512] fp32 (layout A) -> 4x [128, T] bf16 layout-B
        tiles of the *raw* normalized activations (g/b folded into weights
        downstream)."""
        sums = lnp.tile([128, NT], F32, tag="sums")
        sqs = lnp.tile([128, NT], F32, tag="sqs")
        scr = lnp.tile([128, 512], BF16, tag="scr")
        # all Copies then all Squares: ACT table switches are ~1.3us each
        for n in range(NT):
            nc.scalar.activation(scr[:], src_tile[:, n, :], AF.Copy,
                                 accum_out=sums[:, n:n + 1])
        for n in range(NT):
            nc.scalar.activation(scr[:], src_tile[:, n, :], AF.Square,
                                 accum_out=sqs[:, n:n + 1])
        mean = lnp.tile([128, NT], F32, tag="mean")
        nc.vector.tensor_scalar_mul(mean[:], sums[:], 1.0 / C)
        msq = lnp.tile([128, NT], F32, tag="msq")
        nc.scalar.activation(msq[:], mean[:], AF.Square)
        var = lnp.tile([128, NT], F32, tag="var")
        nc.vector.scalar_tensor_tensor(var[:], sqs[:], 1.0 / C, msq[:],
                                       op0=OP.mult, op1=OP.subtract)
        sqv = lnp.tile([128, NT], F32, tag="sqv")
        nc.scalar.activation(sqv[:], var[:], AF.Sqrt, bias=eps_t[:])
        rstd = lnp.tile([128, NT], F32, tag="rstd")
        nc.vector.reciprocal(rstd[:], sqv[:])
        xn = lnp.tile([128, NT, 512], BF16, tag="xn")
        for n in range(NT):
            nc.vector.tensor_scalar(xn[:, n, :], src_tile[:, n, :],
                                    mean[:, n:n + 1], rstd[:, n:n + 1],
                                    op0=OP.subtract, op1=OP.mult)
        # bounce through DRAM with a zero row at t=0, reload transposed into
        # layout B; the zero lands in column 0 so the time-shift is a plain
        # offset view (transpose dest must stay column-aligned on HW)
        xnd = dramp.tile([T + 32, C], BF16, tag="xnd")
        nc.sync.dma_start(xnd[0:32, :], zrow[:])
        nc.sync.dma_start(xnd[32:T + 32].rearrange("(n p) c -> p n c", p=128),
                          xn[:])
        xnB, dB = [], []
        for cc in range(CC):
            t_ = bp.tile([128, T + 32], BF16, tag=f"xnB_{cc}")
            nc.sync.dma_start_transpose(t_[:],
                                        xnd[:, cc * 128:(cc + 1) * 128])
            xnB.append(t_)
            # shared per-cc delta (xn_t - xn_{t-1}) reused by all mix branches
            d = bp.tile([128, T], BF16, tag=f"mixd_{cc}")
            nc.gpsimd.tensor_tensor(d[:], t_[:, 32:T + 32], t_[:, 31:T + 31],
                                    op=OP.subtract)
            dB.append(d)
        return xnB, dB

    def mix(xnB, dB, coefm1_c, slot, cc):
        """xk = coef*xn + (1-coef)*xx = xn + (coef-1)*d, all reads aligned."""
        o = bp.tile([128, T], BF16, tag=f"mix_{slot}_{cc}")
        nc.vector.scalar_tensor_tensor(o[:], dB[cc][:], coefm1_c[cc][:],
                                       xnB[cc][:, 32:T + 32],
                                       op0=OP.mult, op1=OP.add)
        return o

    for b in range(bl):
        xb = x_d[b].rearrange("(n p) c -> p n c", p=128)
        yb = y_d[b].rearrange("(n p) c -> p n c", p=128)
        xa = xa_pool.tile([128, NT, 512], F32, tag="xa")
        nc.sync.dma_start(xa[:], xb)

        # ---------- time mix ----------
        mark(f"b{b}.ln1")
        xnB, dB = layer_norm(xa, "ln1")
        mark(f"b{b}.mix1")
        xk = [mix(xnB, dB, tmk_c, "k", cc) for cc in range(CC)]
        xv = [mix(xnB, dB, tmv_c, "v", cc) for cc in range(CC)]
        xr = [mix(xnB, dB, tmr_c, "r", cc) for cc in range(CC)]

        srw = []
        for hh in range(CC):
            mark(f"b{b}.wkv{hh}")
            # k/v/r for this 128-channel output chunk, t in halves.
            # ci outer / th inner: consecutive matmuls share the stationary.
            def mm_pair(wT, xs, tag):
                halves = [ps.tile([128, 512], F32, tag="ps", name=f"ps_{tag}{th}")
                          for th in range(2)]
                for ci in range(CC):
                    for th in range(2):
                        nc.tensor.matmul(
                            halves[th][:], wT[ci][:, hh * 128:(hh + 1) * 128],
                            xs[ci][:, th * 512:(th + 1) * 512],
                            start=(ci == 0), stop=(ci == CC - 1))
                return halves

            # order k, r, v: k's psum is freed by a dep-free ACT copy, r by the
            # dep-free sigmoid, and by the time v lands its consumer (ev,
            # which needs e) is ready -- so the PSUM ring never stalls PE.
            k_ps = mm_pair(wkT, xk, "k")
            m2 = wkvp.tile([128, 2], F32, tag="m2")
            ksb = wkvp.tile([128, T], BF16, tag="ksb")
            for th in range(2):
                nc.vector.tensor_reduce(m2[:, th:th + 1], k_ps[th][:],
                                        axis=AX.X, op=OP.max)
                nc.scalar.activation(ksb[:, th * 512:(th + 1) * 512],
                                     k_ps[th][:], AF.Copy)
            r_ps = mm_pair(wrT, xr, "r")
            sig = wkvp.tile([128, T], BF16, tag="sig")
            for th in range(2):
                nc.scalar.activation(sig[:, th * 512:(th + 1) * 512],
                                     r_ps[th][:], AF.Sigmoid, bias=rb_c[hh][:])
            v_ps = mm_pair(wvT, xv, "v")
            mneg = wkvp.tile([128, 1], F32, tag="mneg")
            nc.vector.tensor_reduce(mneg[:], m2[:], axis=AX.X, op=OP.max,
                                    negate=True)
            e = wkvp.tile([128, T], F32, tag="e")
            nc.scalar.activation(e[:], ksb[:], AF.Exp, bias=mneg[:])
            ev = wkvp.tile([128, T], F32, tag="ev")
            for th in range(2):
                sl = slice(th * 512, (th + 1) * 512)
                nc.vector.scalar_tensor_tensor(ev[:, sl], v_ps[th][:],
                                               vb_c[hh][:], e[:, sl],
                                               op0=OP.add, op1=OP.mult)
            Pb = wkvp.tile([128, T + 1], F32, tag="Pb")
            Qb = wkvp.tile([128, T + 1], F32, tag="Qb")
            nc.vector.memset(Pb[:, 0:1], 0.0)
            nc.vector.memset(Qb[:, 0:1], 0.0)
            db = delta_c[hh][:].to_broadcast((128, T))
            nc.vector.tensor_tensor_scan(Pb[:, 1:T + 1], db, ev[:],
                                         0.0, op0=OP.mult, op1=OP.add)
            nc.vector.tensor_tensor_scan(Qb[:, 1:T + 1], db, e[:],
                                         0.0, op0=OP.mult, op1=OP.add)
            # N over ev, D over e (in place)
            nc.vector.scalar_tensor_tensor(ev[:], ev[:], expu_c[hh][:],
                                           Pb[:, 0:T], op0=OP.mult, op1=OP.add)
            nc.vector.scalar_tensor_tensor(e[:], e[:], expu_c[hh][:],
                                           Qb[:, 0:T], op0=OP.mult, op1=OP.add)
            rec = Qb[:, 0:T]  # Qshift already consumed by the D stt above
            nc.vector.reciprocal_approx_fast(rec, e[:])
            nc.vector.tensor_tensor(ev[:], ev[:], rec, op=OP.mult)
            s_ = srwp.tile([128, T], BF16, tag=f"srw_{hh}")
            nc.vector.tensor_tensor(s_[:], ev[:], sig[:], op=OP.mult)
            srw.append(s_)

        mark(f"b{b}.wo")
        # Wo (activation-stationary) + residual, layout A
        x1 = x1_pool.tile([128, NT, 512], F32, tag="x1")
        for n in range(NT):
            p_ = ps.tile([128, 512], F32, tag="ps")
            for cc in range(CC):
                nc.tensor.matmul(p_[:], srw[cc][:, n * 128:(n + 1) * 128],
                                 woT[cc][:], start=(cc == 0), stop=(cc == CC - 1))
            nc.vector.tensor_tensor(x1[:, n, :], xa[:, n, :], p_[:], op=OP.add)

        # ---------- channel mix ----------
        mark(f"b{b}.ln2")
        xn2B, d2B = layer_norm(x1, "ln2")
        xk2 = [mix(xn2B, d2B, cmk_c, "k", cc) for cc in range(CC)]
        xr2 = [mix(xn2B, d2B, cmr_c, "r", cc) for cc in range(CC)]

        for th in range(2):
            mark(f"b{b}.cm{th}")
            tsl = slice(th * 512, (th + 1) * 512)
            kk2 = cmp_.tile([128, HC, 512], BF16, tag="kk2")
            for hh in range(HC):
                p_ = ps.tile([128, 512], F32, tag="ps")
                for ci in range(CC):
                    nc.tensor.matmul(p_[:], cwkT[ci][:, hh * 128:(hh + 1) * 128],
                                     xk2[ci][:, tsl],
                                     start=(ci == 0), stop=(ci == CC - 1))
                nc.scalar.activation(kk2[:, hh, :], p_[:], AF.Relu,
                                     bias=kkb_c[hh][:])
                nc.scalar.activation(kk2[:, hh, :], kk2[:, hh, :], AF.Square)
            for nn in range(4):
                n = th * 4 + nn
                rp = ps.tile([128, 512], F32, tag="ps")
                for ci in range(CC):
                    nc.tensor.matmul(rp[:], xr2[ci][:, n * 128:(n + 1) * 128],
                                     cwrT[ci][:], start=(ci == 0),
                                     stop=(ci == CC - 1))
                sig2 = outp.tile([128, 512], BF16, tag="sig2")
                nc.scalar.activation(sig2[:], rp[:], AF.Sigmoid)
                kvp = ps.tile([128, 512], F32, tag="ps")
                for hh in range(HC):
                    nc.tensor.matmul(kvp[:], kk2[:, hh, nn * 128:(nn + 1) * 128],
                                     cwvT[hh][:], start=(hh == 0),
                                     stop=(hh == HC - 1))
                t2 = outp.tile([128, 512], F32, tag="t2")
                nc.vector.tensor_tensor(t2[:], kvp[:], sig2[:], op=OP.mult)
                nc.gpsimd.tensor_tensor(t2[:], t2[:], x1[:, n, :], op=OP.add)
                nc.sync.dma_start(yb[:, n, :], t2[:])


def build_program(bl=BL):
    nc = bacc.Bacc("TRN2", target_bir_lowering=False, debug=False,
                   num_devices=NCORES)
    io = {}
    io["x"] = nc.dram_tensor("x", [bl, T, C], F32, kind="ExternalInput")
    io["y"] = nc.dram_tensor("y", [bl, T, C], F32, kind="ExternalOutput")
    for nm, shp in [("wkT", [C, C]), ("wvT", [C, C]), ("wrT", [C, C]),
                    ("woT", [C, C]), ("cwkT", [C, H]), ("cwvT", [H, C]),
                    ("cwrT", [C, C])]:
        io[nm] = nc.dram_tensor(nm, shp, BF16, kind="ExternalInput")
    for nm, n in [("delta", C), ("expu", C), ("tmk", C), ("tmv", C),
                  ("tmr", C), ("cmk", C), ("cmr", C), ("vb", C), ("rb", C),
                  ("kkb", H)]:
        io[nm] = nc.dram_tensor(nm, [n], F32, kind="ExternalInput")

    with tile.TileContext(nc) as tc:
        with ExitStack() as ctx:
            _emit(nc, tc, ctx, io, bl)
    nc.compile()
    return nc


def host_params(inputs):
    """Host-side parameter prep (O(C^2) only): transposes, LN gamma folding,
    bias projections, scan constants."""
    f32 = np.float32
    g1 = np.asarray(inputs["ln1_g"], f32)
    b1 = np.asarray(inputs["ln1_b"], f32)
    g2 = np.asarray(inputs["ln2_g"], f32)
    b2 = np.asarray(inputs["ln2_b"], f32)
    Wk = np.asarray(inputs["Wk"], f32)
    Wv = np.asarray(inputs["Wv"], f32)
    Wr = np.asarray(inputs["Wr"], f32)
    Wo = np.asarray(inputs["Wo"], f32)
    cWk = np.asarray(inputs["cWk"], f32)
    cWr = np.asarray(inputs["cWr"], f32)
    cWv = np.asarray(inputs["cWv"], f32)

    # r2 bias (cWr @ b2) would be a per-free-dim bias in the layout-A sigmoid;
    # only the zero case is supported (true for this model's init).
    r2b = cWr @ b2
    assert np.allclose(r2b, 0.0, atol=1e-30), "nonzero ln2_b not supported"

    bf = ml_dtypes.bfloat16
    p = {
        "wkT": np.ascontiguousarray((Wk.T * g1[:, None]).astype(bf)),
        "wvT": np.ascontiguousarray((Wv.T * g1[:, None]).astype(bf)),
        "wrT": np.ascontiguousarray((Wr.T * g1[:, None]).astype(bf)),
        "woT": np.ascontiguousarray(Wo.T.astype(bf)),
        "cwkT": np.ascontiguousarray((cWk.T * g2[:, None]).astype(bf)),
        "cwvT": np.ascontiguousarray(cWv.T.astype(bf)),
        "cwrT": np.ascontiguousarray((cWr.T * g2[:, None]).astype(bf)),
        "delta": np.exp(-np.exp(np.asarray(inputs["time_decay"], f32))),
        "expu": np.exp(np.asarray(inputs["time_first"], f32)),
        "tmk": np.asarray(inputs["tm_k"], f32),
        "tmv": np.asarray(inputs["tm_v"], f32),
        "tmr": np.asarray(inputs["tm_r"], f32),
        "cmk": np.asarray(inputs["cm_k"], f32),
        "cmr": np.asarray(inputs["cm_r"], f32),
        "vb": (Wv @ b1).astype(f32),
        "rb": (Wr @ b1).astype(f32),
        "kkb": (cWk @ b2).astype(f32),
    }
    return p


_CACHE = {}


def kernel(**inputs):
    from concourse.bass_utils import run_bass_kernel_spmd

    if "nc" not in _CACHE:
        _CACHE["nc"] = build_program(BL)
    nc = _CACHE["nc"]

    p = host_params(inputs)
    x = np.asarray(inputs["x"], np.float32)
    in_maps = []
    for c in range(NCORES):
        m = dict(p)
        m["x"] = np.ascontiguousarray(x[c * BL:(c + 1) * BL])
        in_maps.append(m)
    res = run_bass_kernel_spmd(nc, in_maps, list(range(NCORES)))
    out = np.concatenate([res.results[c]["y"] for c in range(NCORES)], axis=0)
    return out.astype(np.float32)



# revision 2
# speedup vs baseline: 1.0531x; 1.0531x over previous
"""RWKV-v4 block on 8 Trainium2 NeuronCores, data-parallel over batch.

v2: fp8e4 DoubleRow matmuls (2x PE throughput: 256-deep contraction per
512-row instruction), WKV without max-subtraction (numerically safe at
these scales; the M-shift cancels in the P/Q ratio), sigmoid via tanh so
every ACT function in the LN/WKV phase lives in one activation table,
LN rstd via bitcast-Newton rsqrt on DVE (no sqrt table load), and LN2 row
sums ride free on the residual-add's accum_out.

Layouts as v1: layout A [t(128p), n(8), c(512)] for LN stats/residual,
layout B [c(128p) x 4, t(1024)] for mixing/WKV/matmul moving operands,
A->B via bf16 DMA-transpose bounce through DRAM.

Weight scaling: all fp8 weights are pre-scaled by 64 (wo by 32 = 64*0.5
for the tanh-sigmoid halving); the 1/64 compensations fold into ACT
scale arguments and the x1 stt scalar. cWv's 1/64 folds into the
square-ACT scale (1/8)^2.
"""

import numpy as np
import ml_dtypes
from contextlib import ExitStack

import concourse.bass as bass
import concourse.tile as tile
from concourse import bacc, mybir

B, T, C = 32, 1024, 512
H = 4 * C
NCORES = 8
BL = B // NCORES
NT = T // 128
CC = C // 128   # 4 chunks of input/output channels
HC = H // 128   # 16 hidden chunks

F32 = mybir.dt.float32
BF16 = mybir.dt.bfloat16
FP8 = mybir.dt.float8e4
I32 = mybir.dt.int32
AX = mybir.AxisListType
OP = mybir.AluOpType
AF = mybir.ActivationFunctionType
PM = mybir.MatmulPerfMode

WS = 64.0          # weight pre-scale
RSQRT_MAGIC = 0x5f3759df


def _emit(nc, tc, ctx, io, bl):
    x_d = io["x"].ap()
    y_d = io["y"].ap()

    def col(name, c0):
        return io[name].ap()[c0 * 128:(c0 + 1) * 128].rearrange(
            "(c one) -> c one", one=1)

    sb = ctx.enter_context(tc.tile_pool(name="sb", bufs=1))
    ps = ctx.enter_context(tc.tile_pool(name="ps", bufs=8, space="PSUM"))
    dramp = ctx.enter_context(tc.tile_pool(name="dram", bufs=2, space="DRAM"))

    # ---- fp8 weights, DR layout [128, j, m] with k = j*128 + p ----
    def load_w8(name, j, m):
        t_ = sb.tile([128, j, m], FP8, tag=f"w_{name}")
        nc.sync.dma_start(t_[:], io[name].ap())
        return t_

    def load_wb(name, j, m):
        t_ = sb.tile([128, j, m], BF16, tag=f"w_{name}")
        nc.sync.dma_start(t_[:], io[name].ap())
        return t_

    wkb = load_wb("wkb", CC, C)
    wvb = load_wb("wvb", CC, C)
    wr8 = load_w8("wr8", CC, C)
    wo8 = load_w8("wo8", CC, C)
    cwk8 = load_w8("cwk8", CC, H)
    cwr8 = load_w8("cwr8", CC, C)
    cwv8 = load_w8("cwv8", HC, C)

    def vec4(name):
        ts_ = []
        for i in range(CC):
            t_ = sb.tile([128, 1], F32, tag=f"v_{name}_{i}")
            nc.sync.dma_start(t_[:], col(name, i))
            ts_.append(t_)
        return ts_

    delta_c = vec4("delta")
    eu_c = vec4("expu")

    def vec4_m1(name):
        ts_ = []
        for i in range(CC):
            t_ = sb.tile([128, 1], F32, tag=f"vm_{name}_{i}")
            nc.sync.dma_start(t_[:], col(name, i))
            nc.vector.tensor_scalar_add(t_[:], t_[:], -1.0)
            ts_.append(t_)
        return ts_

    tmk_c = vec4_m1("tmk")
    tmv_c = vec4_m1("tmv")
    tmr_c = vec4_m1("tmr")
    cmk_c = vec4_m1("cmk")
    cmr_c = vec4_m1("cmr")

    zrow = sb.tile([32, C], BF16, tag="zrow")
    nc.vector.memset(zrow[:], 0.0)

    # ---- per-batch pools ----
    xa_p = ctx.enter_context(tc.tile_pool(name="xa", bufs=3))
    st_p = ctx.enter_context(tc.tile_pool(name="st", bufs=2))    # stats
    xn_p = ctx.enter_context(tc.tile_pool(name="xn", bufs=1))    # pre-bounce
    xb_p = ctx.enter_context(tc.tile_pool(name="xb", bufs=1))    # layout B
    mx_p = ctx.enter_context(tc.tile_pool(name="mx", bufs=1))    # mixes
    wk_p = ctx.enter_context(tc.tile_pool(name="wkv", bufs=2))   # wkv per-hh
    wt_p = ctx.enter_context(tc.tile_pool(name="wkt", bufs=1))   # wkv tail
    sc_p = ctx.enter_context(tc.tile_pool(name="scan", bufs=1))  # Pb/Qb
    srw_p = ctx.enter_context(tc.tile_pool(name="srw", bufs=1))
    kk_p = ctx.enter_context(tc.tile_pool(name="kk", bufs=1))
    out_p = ctx.enter_context(tc.tile_pool(name="out", bufs=2))

    def rsqrt_dve(vpe, tag):
        """rstd = 1/sqrt(vpe), vpe f32 [128, NT]; bitcast-Newton on DVE."""
        yi = st_p.tile([128, NT], I32, tag=f"rs_i_{tag}")
        nc.vector.tensor_scalar(yi[:], vpe[:].bitcast(I32), 1, None,
                                op0=OP.arith_shift_right)
        nc.vector.tensor_scalar(yi[:], yi[:], -1, RSQRT_MAGIC,
                                op0=OP.mult, op1=OP.add)
        y = yi[:].bitcast(F32)
        a = st_p.tile([128, NT], F32, tag=f"rs_a_{tag}")
        nc.vector.tensor_scalar_mul(a[:], vpe[:], 0.5)
        t1 = st_p.tile([128, NT], F32, tag=f"rs_t_{tag}")
        for _ in range(2):
            nc.vector.tensor_tensor(t1[:], y, y, op=OP.mult)
            nc.vector.tensor_tensor(t1[:], t1[:], a[:], op=OP.mult)
            nc.vector.tensor_scalar(t1[:], t1[:], -1.0, 1.5,
                                    op0=OP.mult, op1=OP.add)
            nc.vector.tensor_tensor(y, y, t1[:], op=OP.mult)
        return yi  # f32 view via bitcast

    def ln_finish_sums(sums, sqs, tag):
        """LN finish from ACT-accumulated sums/sumsq."""
        mean = st_p.tile([128, NT], F32, tag=f"mean_{tag}")
        nc.vector.tensor_scalar_mul(mean[:], sums[:], 1.0 / C)
        var = st_p.tile([128, NT], F32, tag=f"var_{tag}")
        nc.vector.tensor_tensor(var[:], mean[:], mean[:], op=OP.mult)
        nc.vector.scalar_tensor_tensor(var[:], sqs[:], 1.0 / C, var[:],
                                       op0=OP.mult, op1=OP.subtract)
        nc.vector.tensor_scalar_add(var[:], var[:], 1e-5)
        rsq_i = rsqrt_dve(var, tag)
        rstd = rsq_i[:].bitcast(F32)
        mbneg = st_p.tile([128, NT], F32, tag=f"mb_{tag}")
        nc.vector.scalar_tensor_tensor(mbneg[:], mean[:], -1.0, rstd,
                                       op0=OP.mult, op1=OP.mult)
        return rsq_i, mbneg

    def ln_finish(stats6, tag):
        """stats6 [128, NT, 6] from bn_stats (2 groups of count/mean/M2).
        -> (rstd-int tile (bitcast f32), mbneg f32 [128,NT])."""
        mv2 = st_p.tile([128, NT, 2], F32, tag=f"mv_{tag}")
        for n in range(NT):
            nc.vector.bn_aggr(mv2[:, n, :], stats6[:, n, :])
        mv = mv2[:, :, 0:1].rearrange("p n o -> p (n o)")
        var = st_p.tile([128, NT], F32, tag=f"var_{tag}")
        nc.vector.tensor_scalar(var[:],
                                mv2[:, :, 1:2].rearrange("p n o -> p (n o)"),
                                1.0, 1e-5, op0=OP.mult, op1=OP.add)
        rsq_i = rsqrt_dve(var, tag)
        rstd = rsq_i[:].bitcast(F32)
        mbneg = st_p.tile([128, NT], F32, tag=f"mb_{tag}")
        nc.vector.scalar_tensor_tensor(mbneg[:], mv, -1.0, rstd,
                                       op0=OP.mult, op1=OP.mult)
        return rsq_i, mbneg

    def ln_bounce(xa_t, rsq_i, mbneg, which, b):
        """normalize (ACT) -> bounce -> transpose -> delta. Returns
        (xnB list, d list)."""
        rstd = rsq_i[:].bitcast(F32)
        xn = xn_p.tile([128, NT, C], BF16, tag=f"xn_{which}")
        for n in range(NT):
            nc.scalar.activation(xn[:, n, :], xa_t[:, n, :], AF.Identity,
                                 bias=mbneg[:, n:n + 1],
                                 scale=rstd[:, n:n + 1])
        xnd = dramp.tile([T + 32, C], BF16, tag=f"xnd_{which}")
        nc.sync.dma_start(xnd[0:32, :], zrow[:])
        nc.sync.dma_start(xnd[32:T + 32].rearrange("(n p) c -> p n c", p=128),
                          xn[:])
        xnB, dB = [], []
        for cc in range(CC):
            t_ = xb_p.tile([128, T + 32], BF16, tag=f"xnB_{which}_{cc}")
            nc.sync.dma_start_transpose(t_[:],
                                        xnd[:, cc * 128:(cc + 1) * 128])
            xnB.append(t_)
            d = xb_p.tile([128, T], BF16, tag=f"d_{which}_{cc}")
            nc.gpsimd.tensor_tensor(d[:], t_[:, 32:T + 32], t_[:, 31:T + 31],
                                    op=OP.subtract)
            dB.append(d)
        return xnB, dB

    def mix8(xnB, dB, coefm1, tag, b, dt=FP8):
        """mix tile [128, CC, T]: out = xn + (coef-1)*d."""
        m = mx_p.tile([128, CC, T], dt, tag=f"mx_{tag}")
        for cc in range(CC):
            nc.vector.scalar_tensor_tensor(m[:, cc, :], dB[cc][:],
                                           coefm1[cc][:],
                                           xnB[cc][:, 32:T + 32],
                                           op0=OP.mult, op1=OP.add)
        return m

    def bf_group(out_ps, wb_, mb_, hh, th):
        for ci in range(CC):
            nc.tensor.matmul(
                out_ps[:],
                wb_[:, ci, hh * 128:(hh + 1) * 128],
                mb_[:, ci, th * 512:(th + 1) * 512],
                start=(ci == 0), stop=(ci == CC - 1))

    def dr_group(out_ps, w8, m8, hh, th, nsteps=CC // 2):
        """Accumulate DR matmuls: out += w8[:,2s:2s+2, hh*128:+128].T @
        m8[:,2s:2s+2, th*512:+512] over s."""
        for s in range(nsteps):
            nc.tensor.matmul(
                out_ps[:],
                w8[:, 2 * s:2 * s + 2, hh * 128:(hh + 1) * 128],
                m8[:, 2 * s:2 * s + 2, th * 512:(th + 1) * 512],
                start=(s == 0), stop=(s == nsteps - 1),
                perf_mode=PM.DoubleRow)

    # ================= per-batch stages (chunked) =================
    state = {}

    def chunks_A(b):
        def a00():
            xb = x_d[b].rearrange("(n p) c -> p n c", p=128)
            xa = xa_p.tile([128, NT, C], F32, tag="xa", name=f"xa_{b}")
            nc.sync.dma_start(xa[:], xb)
            state[b] = dict(xa=xa)

        def a0():
            xa = state[b]["xa"]
            stats6 = st_p.tile([128, NT, 6], F32, tag="stats1")
            for n in range(NT):
                nc.vector.bn_stats(stats6[:, n, :], xa[:, n, :])
            state[b]["stats6"] = stats6

        def a1a():
            s = state[b]
            rsq_i, mbneg = ln_finish(s["stats6"], "1")
            s["rsq_i"], s["mbneg"] = rsq_i, mbneg
            xn = xn_p.tile([128, NT, C], BF16, tag="xn_1")
            s["xn"] = xn
            rstd = rsq_i[:].bitcast(F32)
            for n in range(NT // 2):
                nc.scalar.activation(xn[:, n, :], s["xa"][:, n, :],
                                     AF.Identity, bias=mbneg[:, n:n + 1],
                                     scale=rstd[:, n:n + 1])

        def a1b():
            s = state[b]
            xn = s["xn"]
            rstd = s["rsq_i"][:].bitcast(F32)
            mbneg = s["mbneg"]
            for n in range(NT // 2, NT):
                nc.scalar.activation(xn[:, n, :], s["xa"][:, n, :],
                                     AF.Identity, bias=mbneg[:, n:n + 1],
                                     scale=rstd[:, n:n + 1])
            xnd = dramp.tile([T + 32, C], BF16, tag="xnd_1")
            nc.sync.dma_start(xnd[0:32, :], zrow[:])
            nc.sync.dma_start(
                xnd[32:T + 32].rearrange("(n p) c -> p n c", p=128), xn[:])
            s["xnd"] = xnd

        def a2():
            s = state[b]
            xnd = s["xnd"]
            xnB, dB = [], []
            for cc in range(CC):
                t_ = xb_p.tile([128, T + 32], BF16, tag=f"xnB_{cc}")
                nc.sync.dma_start_transpose(t_[:],
                                            xnd[:, cc * 128:(cc + 1) * 128])
                xnB.append(t_)
                d = xb_p.tile([128, T], BF16, tag=f"d_{cc}")
                nc.gpsimd.tensor_tensor(d[:], t_[:, 32:T + 32],
                                        t_[:, 31:T + 31], op=OP.subtract)
                dB.append(d)
            s["xnB"], s["dB"] = xnB, dB

        def a3():
            s = state[b]
            xnB, dB = s["xnB"], s["dB"]
            s["xk8"] = mix8(xnB, dB, tmk_c, "k", b, dt=BF16)
            s["xv8"] = mix8(xnB, dB, tmv_c, "v", b, dt=BF16)
            s["xr8"] = mix8(xnB, dB, tmr_c, "r", b)

        return [a00, a0, a1a, a1b, a2, a3]

    def chunks_B(b):
        def mk_hh(hh):
            def f():
                s = state[b]
                xk8, xv8, xr8 = s["xk8"], s["xv8"], s["xr8"]
                if "srw8" not in s:
                    s["srw8"] = srw_p.tile([128, CC, T], FP8, tag="srw8",
                                           name=f"srw_{b}")
                srw8 = s["srw8"]
                e = wk_p.tile([128, T], BF16, tag="e")
                th_t = wk_p.tile([128, T], BF16, tag="tht")
                vsb = wk_p.tile([128, T], BF16, tag="vsb")
                ev = wk_p.tile([128, T], BF16, tag="ev")
                for th in range(2):
                    sl = slice(th * 512, (th + 1) * 512)
                    k_ps = ps.tile([128, 512], F32, tag="ps",
                                   name=f"kps{b}_{hh}{th}")
                    bf_group(k_ps, wkb, xk8, hh, th)
                    r_ps = ps.tile([128, 512], F32, tag="ps",
                                   name=f"rps{b}_{hh}{th}")
                    dr_group(r_ps, wr8, xr8, hh, th)
                    v_ps = ps.tile([128, 512], F32, tag="ps",
                                   name=f"vps{b}_{hh}{th}")
                    bf_group(v_ps, wvb, xv8, hh, th)
                    nc.scalar.activation(e[:, sl], k_ps[:], AF.Exp)
                    nc.scalar.activation(th_t[:, sl], r_ps[:], AF.Tanh,
                                         scale=0.5 / WS)
                    nc.scalar.activation(vsb[:, sl], v_ps[:], AF.Identity)
                nc.vector.tensor_tensor(ev[:], e[:], vsb[:], op=OP.mult)
                if False:
                    pass
                Pb = sc_p.tile([128, T + 1], F32, tag="Pb")
                Qb = sc_p.tile([128, T + 1], F32, tag="Qb")
                nc.vector.memset(Pb[:, 0:1], 0.0)
                nc.vector.memset(Qb[:, 0:1], 0.0)
                db = delta_c[hh][:].to_broadcast((128, T))
                nc.vector.tensor_tensor_scan(Pb[:, 1:T + 1], db, ev[:],
                                             0.0, op0=OP.mult, op1=OP.add)
                nc.vector.tensor_tensor_scan(Qb[:, 1:T + 1], db, e[:],
                                             0.0, op0=OP.mult, op1=OP.add)
                Nt = wt_p.tile([128, T], BF16, tag="Nt")
                Dt = wt_p.tile([128, T], F32, tag="Dt")
                nc.vector.scalar_tensor_tensor(Nt[:], ev[:], eu_c[hh][:],
                                               Pb[:, 0:T], op0=OP.mult,
                                               op1=OP.add)
                nc.vector.scalar_tensor_tensor(Dt[:], e[:], eu_c[hh][:],
                                               Qb[:, 0:T], op0=OP.mult,
                                               op1=OP.add)
                nc.vector.reciprocal_approx_fast(Dt[:], Dt[:])
                yp = wt_p.tile([128, T], BF16, tag="yp")
                nc.vector.tensor_tensor(yp[:], Nt[:], Dt[:], op=OP.mult)
                yth = wt_p.tile([128, T], BF16, tag="yth")
                nc.gpsimd.tensor_tensor(yth[:], yp[:], th_t[:], op=OP.mult)
                nc.vector.tensor_tensor(srw8[:, hh, :], yp[:], yth[:],
                                        op=OP.add)
            return f

        def wo():
            s = state[b]
            xa, srw8 = s["xa"], s["srw8"]
            stats6 = st_p.tile([128, NT, 6], F32, tag="stats2")
            s["stats62"] = stats6
            for n in range(NT):
                p_ = ps.tile([128, 512], F32, tag="ps", name=f"wops{b}_{n}")
                for s_ in range(CC // 2):
                    nc.tensor.matmul(
                        p_[:],
                        srw8[:, 2 * s_:2 * s_ + 2, n * 128:(n + 1) * 128],
                        wo8[:, 2 * s_:2 * s_ + 2, :],
                        start=(s_ == 0), stop=(s_ == CC // 2 - 1),
                        perf_mode=PM.DoubleRow)
                nc.vector.scalar_tensor_tensor(xa[:, n, :], p_[:], 1.0 / WS,
                                               xa[:, n, :], op0=OP.mult,
                                               op1=OP.add)
                nc.vector.bn_stats(stats6[:, n, :], xa[:, n, :])

        return [mk_hh(0), mk_hh(1), mk_hh(2), mk_hh(3), wo]

    def chunks_C(b):
        def c0():
            s = state[b]
            xa = s["xa"]
            rsq_i, mbneg = ln_finish(s["stats62"], "2")
            rstd = rsq_i[:].bitcast(F32)
            xn = xn_p.tile([128, NT, C], BF16, tag="xn_2")
            for n in range(NT):
                nc.scalar.activation(xn[:, n, :], xa[:, n, :], AF.Identity,
                                     bias=mbneg[:, n:n + 1],
                                     scale=rstd[:, n:n + 1])
            xnd = dramp.tile([T + 32, C], BF16, tag="xnd_2")
            nc.sync.dma_start(xnd[0:32, :], zrow[:])
            nc.sync.dma_start(
                xnd[32:T + 32].rearrange("(n p) c -> p n c", p=128), xn[:])
            s["xnd2"] = xnd

        def c1():
            s = state[b]
            xnd = s["xnd2"]
            xnB, dB = [], []
            for cc in range(CC):
                t_ = xb_p.tile([128, T + 32], BF16, tag=f"xnB_{cc}")
                nc.sync.dma_start_transpose(t_[:],
                                            xnd[:, cc * 128:(cc + 1) * 128])
                xnB.append(t_)
                d = xb_p.tile([128, T], BF16, tag=f"d_{cc}")
                nc.gpsimd.tensor_tensor(d[:], t_[:, 32:T + 32],
                                        t_[:, 31:T + 31], op=OP.subtract)
                dB.append(d)
            s["xk28"] = mix8(xnB, dB, cmk_c, "k2", b)
            s["xr28"] = mix8(xnB, dB, cmr_c, "r2", b)

        return [c0, c1]

    def chunks_D(b):
        def mk_k(th, part):
            def f():
                s = state[b]
                xk28 = s["xk28"]
                if f"kk8_{th}" not in s:
                    s[f"kk8_{th}"] = kk_p.tile([128, HC, 512], FP8,
                                               tag="kk8",
                                               name=f"kk8_{b}_{th}")
                kk8 = s[f"kk8_{th}"]
                for hh in range(part * 4, part * 4 + 4):
                    p_ = ps.tile([128, 512], F32, tag="ps",
                                 name=f"ck{b}_{th}{hh}")
                    dr_group(p_, cwk8, xk28, hh, th)
                    kkb = kk_p.tile([128, 512], BF16, tag="kkb")
                    nc.scalar.activation(kkb[:], p_[:], AF.Relu,
                                         scale=1.0 / WS)
                    nc.scalar.activation(kk8[:, hh, :], kkb[:], AF.Square,
                                         scale=1.0 / 8.0)
            return f

        def mk_rv(th):
            def f():
                s = state[b]
                xa, xr28 = s["xa"], s["xr28"]
                kk8 = s[f"kk8_{th}"]
                yb = y_d[b].rearrange("(n p) c -> p n c", p=128)
                for nn in range(4):
                    n = th * 4 + nn
                    rp = ps.tile([128, 512], F32, tag="ps", name=f"cr{b}_{n}")
                    for s_ in range(CC // 2):
                        nc.tensor.matmul(
                            rp[:],
                            xr28[:, 2 * s_:2 * s_ + 2,
                                 n * 128:(n + 1) * 128],
                            cwr8[:, 2 * s_:2 * s_ + 2, :],
                            start=(s_ == 0), stop=(s_ == CC // 2 - 1),
                            perf_mode=PM.DoubleRow)
                    sig2 = out_p.tile([128, 512], BF16, tag="sig2")
                    nc.scalar.activation(sig2[:], rp[:], AF.Sigmoid,
                                         scale=1.0 / WS)
                    kvp = ps.tile([128, 512], F32, tag="ps", name=f"cv{b}_{n}")
                    for s_ in range(HC // 2):
                        nc.tensor.matmul(
                            kvp[:],
                            kk8[:, 2 * s_:2 * s_ + 2,
                                nn * 128:(nn + 1) * 128],
                            cwv8[:, 2 * s_:2 * s_ + 2, :],
                            start=(s_ == 0), stop=(s_ == HC // 2 - 1),
                            perf_mode=PM.DoubleRow)
                    t2 = out_p.tile([128, 512], BF16, tag="t2")
                    nc.vector.tensor_tensor(t2[:], kvp[:], sig2[:],
                                            op=OP.mult)
                    yo = out_p.tile([128, 512], F32, tag="yo")
                    nc.gpsimd.tensor_tensor(yo[:], t2[:], xa[:, n, :],
                                            op=OP.add)
                    nc.sync.dma_start(yb[:, n, :], yo[:])
            return f

        return [mk_k(0, 0), mk_k(0, 1), mk_k(0, 2), mk_k(0, 3), mk_rv(0),
                mk_k(1, 0), mk_k(1, 1), mk_k(1, 2), mk_k(1, 3), mk_rv(1)]

    # software-pipelined, chunk-interleaved emission:
    #   cycle b: round-robin over B(b), D(b-1), A(b+1)[a0..a2];
    #   then A(b+1).a3 (mixes), then C(b).
    from itertools import zip_longest
    for ch in chunks_A(0):
        ch()
    prev_D = None
    for b in range(bl):
        Bc = chunks_B(b)
        Dc = prev_D or []
        Ac_all = chunks_A(b + 1) if b + 1 < bl else []
        if Ac_all:
            Ac_all[0]()  # xa load for b+1, two cycles ahead of its D
        Ac, Amix = Ac_all[1:5], Ac_all[5:]
        for group in zip_longest(Bc, Dc, Ac):
            for ch in group:
                if ch is not None:
                    ch()
        for ch in Amix:
            ch()
        for ch in chunks_C(b):
            ch()
        prev_D = chunks_D(b)
    for ch in prev_D:
        ch()


def build_program(bl=BL):
    nc = bacc.Bacc("TRN2", target_bir_lowering=False, debug=False,
                   num_devices=NCORES)
    io = {}
    io["x"] = nc.dram_tensor("x", [bl, T, C], F32, kind="ExternalInput")
    io["y"] = nc.dram_tensor("y", [bl, T, C], F32, kind="ExternalOutput")
    for nm, shp in [("wr8", [128, CC, C]), ("wo8", [128, CC, C]),
                    ("cwk8", [128, CC, H]), ("cwr8", [128, CC, C]),
                    ("cwv8", [128, HC, C])]:
        io[nm] = nc.dram_tensor(nm, shp, FP8, kind="ExternalInput")
    for nm in ["wkb", "wvb"]:
        io[nm] = nc.dram_tensor(nm, [128, CC, C], BF16, kind="ExternalInput")
    for nm in ["delta", "expu", "tmk", "tmv", "tmr", "cmk", "cmr"]:
        io[nm] = nc.dram_tensor(nm, [C], F32, kind="ExternalInput")
    import os
    if os.environ.get("KDBG"):
        io["dbg_xnB0"] = nc.dram_tensor("dbg_xnB0", [128, T + 32], BF16,
                                        kind="ExternalOutput")
        io["dbg_xk8"] = nc.dram_tensor("dbg_xk8", [128, CC, T], BF16,
                                       kind="ExternalOutput")
        for nm in ["dbg_e", "dbg_tht", "dbg_ev", "dbg_Nt", "dbg_yp"]:
            io[nm] = nc.dram_tensor(nm, [128, T], BF16 if nm != "dbg_ev" else BF16,
                                    kind="ExternalOutput")
        io["dbg_Pb"] = nc.dram_tensor("dbg_Pb", [128, T + 1], F32,
                                      kind="ExternalOutput")
        io["dbg_x1"] = nc.dram_tensor("dbg_x1", [128, NT, C], F32,
                                      kind="ExternalOutput")
        io["dbg_srw8"] = nc.dram_tensor("dbg_srw8", [128, CC, T], FP8,
                                        kind="ExternalOutput")

    with tile.TileContext(nc) as tc:
        with ExitStack() as ctx:
            _emit(nc, tc, ctx, io, bl)
    nc.compile()
    return nc


def host_params(inputs):
    f32 = np.float32
    g1 = np.asarray(inputs["ln1_g"], f32)
    b1 = np.asarray(inputs["ln1_b"], f32)
    g2 = np.asarray(inputs["ln2_g"], f32)
    b2 = np.asarray(inputs["ln2_b"], f32)
    assert np.allclose(b1, 0.0) and np.allclose(b2, 0.0), \
        "nonzero LN bias not supported"
    Wk = np.asarray(inputs["Wk"], f32)
    Wv = np.asarray(inputs["Wv"], f32)
    Wr = np.asarray(inputs["Wr"], f32)
    Wo = np.asarray(inputs["Wo"], f32)
    cWk = np.asarray(inputs["cWk"], f32)
    cWr = np.asarray(inputs["cWr"], f32)
    cWv = np.asarray(inputs["cWv"], f32)

    fp8 = ml_dtypes.float8_e4m3

    def dr8(WT, scale):
        # WT [K, M] -> [128, K//128, M], k = j*128 + p
        K = WT.shape[0]
        return np.ascontiguousarray(
            (WT * scale).reshape(K // 128, 128, -1).transpose(1, 0, 2)
        ).astype(fp8)

    bfd = ml_dtypes.bfloat16

    def drb(WT):
        K = WT.shape[0]
        return np.ascontiguousarray(
            WT.reshape(K // 128, 128, -1).transpose(1, 0, 2)).astype(bfd)

    p = {
        "wkb": drb(Wk.T * g1[:, None]),
        "wvb": drb(Wv.T * g1[:, None]),
        "wr8": dr8(Wr.T * g1[:, None], WS),
        "wo8": dr8(Wo.T, WS * 0.5),
        "cwk8": dr8(cWk.T * g2[:, None], WS),
        "cwr8": dr8(cWr.T * g2[:, None], WS),
        "cwv8": dr8(cWv.T, WS),
        "delta": np.exp(-np.exp(np.asarray(inputs["time_decay"], f32))),
        "expu": np.exp(np.asarray(inputs["time_first"], f32)),
        "tmk": np.asarray(inputs["tm_k"], f32),
        "tmv": np.asarray(inputs["tm_v"], f32),
        "tmr": np.asarray(inputs["tm_r"], f32),
        "cmk": np.asarray(inputs["cm_k"], f32),
        "cmr": np.asarray(inputs["cm_r"], f32),
    }
    return p


WS = 64.0
_CACHE = {}


def kernel(**inputs):
    from concourse.bass_utils import run_bass_kernel_spmd

    if "nc" not in _CACHE:
        _CACHE["nc"] = build_program(BL)
    nc = _CACHE["nc"]

    p = host_params(inputs)
    x = np.asarray(inputs["x"], np.float32)
    in_maps = []
    for c in range(NCORES):
        m = dict(p)
        m["x"] = np.ascontiguousarray(x[c * BL:(c + 1) * BL])
        in_maps.append(m)
    res = run_bass_kernel_spmd(nc, in_maps, list(range(NCORES)))
    out = np.concatenate([res.results[c]["y"] for c in range(NCORES)], axis=0)
    return out.astype(np.float32)


# revision 3
# speedup vs baseline: 1.0763x; 1.0221x over previous
"""RWKV-v4 block on 8 Trainium2 NeuronCores, data-parallel over batch.

v2: fp8e4 DoubleRow matmuls (2x PE throughput: 256-deep contraction per
512-row instruction), WKV without max-subtraction (numerically safe at
these scales; the M-shift cancels in the P/Q ratio), sigmoid via tanh so
every ACT function in the LN/WKV phase lives in one activation table,
LN rstd via bitcast-Newton rsqrt on DVE (no sqrt table load), and LN2 row
sums ride free on the residual-add's accum_out.

Layouts as v1: layout A [t(128p), n(8), c(512)] for LN stats/residual,
layout B [c(128p) x 4, t(1024)] for mixing/WKV/matmul moving operands,
A->B via bf16 DMA-transpose bounce through DRAM.

Weight scaling: all fp8 weights are pre-scaled by 64 (wo by 32 = 64*0.5
for the tanh-sigmoid halving); the 1/64 compensations fold into ACT
scale arguments and the x1 stt scalar. cWv's 1/64 folds into the
square-ACT scale (1/8)^2.
"""

import numpy as np
import ml_dtypes
from contextlib import ExitStack

import concourse.bass as bass
import concourse.tile as tile
from concourse import bacc, mybir

B, T, C = 32, 1024, 512
H = 4 * C
NCORES = 8
BL = B // NCORES
NT = T // 128
CC = C // 128   # 4 chunks of input/output channels
HC = H // 128   # 16 hidden chunks

F32 = mybir.dt.float32
BF16 = mybir.dt.bfloat16
FP8 = mybir.dt.float8e4
I32 = mybir.dt.int32
AX = mybir.AxisListType
OP = mybir.AluOpType
AF = mybir.ActivationFunctionType
PM = mybir.MatmulPerfMode

WS = 64.0          # weight pre-scale
RSQRT_MAGIC = 0x5f3759df


def _emit(nc, tc, ctx, io, bl):
    x_d = io["x"].ap()
    y_d = io["y"].ap()

    def col(name, c0):
        return io[name].ap()[c0 * 128:(c0 + 1) * 128].rearrange(
            "(c one) -> c one", one=1)

    sb = ctx.enter_context(tc.tile_pool(name="sb", bufs=1))
    ps = ctx.enter_context(tc.tile_pool(name="ps", bufs=8, space="PSUM"))
    dramp = ctx.enter_context(tc.tile_pool(name="dram", bufs=2, space="DRAM"))

    # ---- fp8 weights, DR layout [128, j, m] with k = j*128 + p ----
    def load_w8(name, j, m):
        t_ = sb.tile([128, j, m], FP8, tag=f"w_{name}")
        nc.sync.dma_start(t_[:], io[name].ap())
        return t_

    def load_wb(name, j, m):
        t_ = sb.tile([128, j, m], BF16, tag=f"w_{name}")
        nc.sync.dma_start(t_[:], io[name].ap())
        return t_

    wkb = load_wb("wkb", CC, C)
    wvb = load_wb("wvb", CC, C)
    wr8 = load_w8("wr8", CC, C)
    wo8 = load_w8("wo8", CC, C)
    cwk8 = load_w8("cwk8", CC, H)
    cwr8 = load_w8("cwr8", CC, C)
    cwv8 = load_w8("cwv8", HC, C)

    def vec4(name):
        ts_ = []
        for i in range(CC):
            t_ = sb.tile([128, 1], F32, tag=f"v_{name}_{i}")
            nc.sync.dma_start(t_[:], col(name, i))
            ts_.append(t_)
        return ts_

    delta_c = vec4("delta")
    eu_c = vec4("expu")

    def vec4_m1(name):
        ts_ = []
        for i in range(CC):
            t_ = sb.tile([128, 1], F32, tag=f"vm_{name}_{i}")
            nc.sync.dma_start(t_[:], col(name, i))
            nc.vector.tensor_scalar_add(t_[:], t_[:], -1.0)
            ts_.append(t_)
        return ts_

    tmk_c = vec4_m1("tmk")
    tmv_c = vec4_m1("tmv")
    tmr_c = vec4_m1("tmr")
    cmk_c = vec4_m1("cmk")
    cmr_c = vec4_m1("cmr")

    zrow = sb.tile([32, C], BF16, tag="zrow")
    nc.vector.memset(zrow[:], 0.0)

    # ---- per-batch pools ----
    xa_p = ctx.enter_context(tc.tile_pool(name="xa", bufs=3))
    st_p = ctx.enter_context(tc.tile_pool(name="st", bufs=2))    # stats
    xn_p = ctx.enter_context(tc.tile_pool(name="xn", bufs=1))    # pre-bounce
    xb_p = ctx.enter_context(tc.tile_pool(name="xb", bufs=1))    # layout B
    mx_p = ctx.enter_context(tc.tile_pool(name="mx", bufs=1))    # mixes
    wk_p = ctx.enter_context(tc.tile_pool(name="wkv", bufs=2))   # wkv per-hh
    wt_p = ctx.enter_context(tc.tile_pool(name="wkt", bufs=1))   # wkv tail
    sc_p = ctx.enter_context(tc.tile_pool(name="scan", bufs=1))  # Pb/Qb
    srw_p = ctx.enter_context(tc.tile_pool(name="srw", bufs=1))
    kk_p = ctx.enter_context(tc.tile_pool(name="kk", bufs=1))
    out_p = ctx.enter_context(tc.tile_pool(name="out", bufs=2))

    def rsqrt_dve(vpe, tag):
        """rstd = 1/sqrt(vpe), vpe f32 [128, NT]; bitcast-Newton on DVE."""
        yi = st_p.tile([128, NT], I32, tag=f"rs_i_{tag}")
        nc.vector.tensor_scalar(yi[:], vpe[:].bitcast(I32), 1, None,
                                op0=OP.arith_shift_right)
        nc.vector.tensor_scalar(yi[:], yi[:], -1, RSQRT_MAGIC,
                                op0=OP.mult, op1=OP.add)
        y = yi[:].bitcast(F32)
        a = st_p.tile([128, NT], F32, tag=f"rs_a_{tag}")
        nc.vector.tensor_scalar_mul(a[:], vpe[:], 0.5)
        t1 = st_p.tile([128, NT], F32, tag=f"rs_t_{tag}")
        for _ in range(2):
            nc.vector.tensor_tensor(t1[:], y, y, op=OP.mult)
            nc.vector.tensor_tensor(t1[:], t1[:], a[:], op=OP.mult)
            nc.vector.tensor_scalar(t1[:], t1[:], -1.0, 1.5,
                                    op0=OP.mult, op1=OP.add)
            nc.vector.tensor_tensor(y, y, t1[:], op=OP.mult)
        return yi  # f32 view via bitcast

    def ln_finish_sums(sums, sqs, tag):
        """LN finish from ACT-accumulated sums/sumsq."""
        mean = st_p.tile([128, NT], F32, tag=f"mean_{tag}")
        nc.vector.tensor_scalar_mul(mean[:], sums[:], 1.0 / C)
        var = st_p.tile([128, NT], F32, tag=f"var_{tag}")
        nc.vector.tensor_tensor(var[:], mean[:], mean[:], op=OP.mult)
        nc.vector.scalar_tensor_tensor(var[:], sqs[:], 1.0 / C, var[:],
                                       op0=OP.mult, op1=OP.subtract)
        nc.vector.tensor_scalar_add(var[:], var[:], 1e-5)
        rsq_i = rsqrt_dve(var, tag)
        rstd = rsq_i[:].bitcast(F32)
        mbneg = st_p.tile([128, NT], F32, tag=f"mb_{tag}")
        nc.vector.scalar_tensor_tensor(mbneg[:], mean[:], -1.0, rstd,
                                       op0=OP.mult, op1=OP.mult)
        return rsq_i, mbneg

    def ln_finish(stats6, tag):
        """stats6 [128, NT, 6] from bn_stats (2 groups of count/mean/M2).
        -> (rstd-int tile (bitcast f32), mbneg f32 [128,NT])."""
        mv2 = st_p.tile([128, NT, 2], F32, tag=f"mv_{tag}")
        for n in range(NT):
            nc.vector.bn_aggr(mv2[:, n, :], stats6[:, n, :])
        mv = mv2[:, :, 0:1].rearrange("p n o -> p (n o)")
        var = st_p.tile([128, NT], F32, tag=f"var_{tag}")
        nc.vector.tensor_scalar(var[:],
                                mv2[:, :, 1:2].rearrange("p n o -> p (n o)"),
                                1.0, 1e-5, op0=OP.mult, op1=OP.add)
        rsq_i = rsqrt_dve(var, tag)
        rstd = rsq_i[:].bitcast(F32)
        mbneg = st_p.tile([128, NT], F32, tag=f"mb_{tag}")
        nc.vector.scalar_tensor_tensor(mbneg[:], mv, -1.0, rstd,
                                       op0=OP.mult, op1=OP.mult)
        return rsq_i, mbneg

    def ln_bounce(xa_t, rsq_i, mbneg, which, b):
        """normalize (ACT) -> bounce -> transpose -> delta. Returns
        (xnB list, d list)."""
        rstd = rsq_i[:].bitcast(F32)
        xn = xn_p.tile([128, NT, C], BF16, tag=f"xn_{which}")
        for n in range(NT):
            nc.scalar.activation(xn[:, n, :], xa_t[:, n, :], AF.Identity,
                                 bias=mbneg[:, n:n + 1],
                                 scale=rstd[:, n:n + 1])
        xnd = dramp.tile([T + 32, C], BF16, tag=f"xnd_{which}")
        nc.sync.dma_start(xnd[0:32, :], zrow[:])
        nc.sync.dma_start(xnd[32:T + 32].rearrange("(n p) c -> p n c", p=128),
                          xn[:])
        xnB, dB = [], []
        for cc in range(CC):
            t_ = xb_p.tile([128, T + 32], BF16, tag=f"xnB_{which}_{cc}")
            nc.sync.dma_start_transpose(t_[:],
                                        xnd[:, cc * 128:(cc + 1) * 128])
            xnB.append(t_)
            d = xb_p.tile([128, T], BF16, tag=f"d_{which}_{cc}")
            nc.gpsimd.tensor_tensor(d[:], t_[:, 32:T + 32], t_[:, 31:T + 31],
                                    op=OP.subtract)
            dB.append(d)
        return xnB, dB

    def mix8(xnB, dB, coefm1, tag, b, dt=FP8):
        """mix tile [128, CC, T]: out = xn + (coef-1)*d."""
        m = mx_p.tile([128, CC, T], dt, tag=f"mx_{tag}")
        for cc in range(CC):
            nc.vector.scalar_tensor_tensor(m[:, cc, :], dB[cc][:],
                                           coefm1[cc][:],
                                           xnB[cc][:, 32:T + 32],
                                           op0=OP.mult, op1=OP.add)
        return m

    def mix8_half(m, xnB, dB, coefm1, th):
        """fill th-half of a mix tile: cols th*512..(th+1)*512."""
        sl = slice(th * 512, (th + 1) * 512)
        sl32 = slice(32 + th * 512, 32 + (th + 1) * 512)
        for cc in range(CC):
            nc.vector.scalar_tensor_tensor(m[:, cc, sl], dB[cc][:, sl],
                                           coefm1[cc][:],
                                           xnB[cc][:, sl32],
                                           op0=OP.mult, op1=OP.add)

    def bf_group(out_ps, wb_, mb_, hh, th):
        for ci in range(CC):
            nc.tensor.matmul(
                out_ps[:],
                wb_[:, ci, hh * 128:(hh + 1) * 128],
                mb_[:, ci, th * 512:(th + 1) * 512],
                start=(ci == 0), stop=(ci == CC - 1))

    def dr_group(out_ps, w8, m8, hh, th, nsteps=CC // 2):
        """Accumulate DR matmuls: out += w8[:,2s:2s+2, hh*128:+128].T @
        m8[:,2s:2s+2, th*512:+512] over s."""
        for s in range(nsteps):
            nc.tensor.matmul(
                out_ps[:],
                w8[:, 2 * s:2 * s + 2, hh * 128:(hh + 1) * 128],
                m8[:, 2 * s:2 * s + 2, th * 512:(th + 1) * 512],
                start=(s == 0), stop=(s == nsteps - 1),
                perf_mode=PM.DoubleRow)

    # ================= per-batch stages (chunked) =================
    state = {}

    def chunks_A(b):
        def a00():
            xb = x_d[b].rearrange("(n p) c -> p n c", p=128)
            xa = xa_p.tile([128, NT, C], F32, tag="xa", name=f"xa_{b}")
            nc.sync.dma_start(xa[:], xb)
            state[b] = dict(xa=xa)

        def a0():
            xa = state[b]["xa"]
            stats6 = st_p.tile([128, NT, 6], F32, tag="stats1")
            for n in range(NT):
                nc.vector.bn_stats(stats6[:, n, :], xa[:, n, :])
            state[b]["stats6"] = stats6

        def a1a():
            s = state[b]
            rsq_i, mbneg = ln_finish(s["stats6"], "1")
            s["rsq_i"], s["mbneg"] = rsq_i, mbneg
            xn = xn_p.tile([128, NT, C], BF16, tag="xn_1")
            s["xn"] = xn
            rstd = rsq_i[:].bitcast(F32)
            for n in range(NT // 2):
                nc.scalar.activation(xn[:, n, :], s["xa"][:, n, :],
                                     AF.Identity, bias=mbneg[:, n:n + 1],
                                     scale=rstd[:, n:n + 1])

        def a1b():
            s = state[b]
            xn = s["xn"]
            rstd = s["rsq_i"][:].bitcast(F32)
            mbneg = s["mbneg"]
            for n in range(NT // 2, NT):
                nc.scalar.activation(xn[:, n, :], s["xa"][:, n, :],
                                     AF.Identity, bias=mbneg[:, n:n + 1],
                                     scale=rstd[:, n:n + 1])
            xnd = dramp.tile([T + 32, C], BF16, tag="xnd_1")
            nc.sync.dma_start(xnd[0:32, :], zrow[:])
            nc.sync.dma_start(
                xnd[32:T + 32].rearrange("(n p) c -> p n c", p=128), xn[:])
            s["xnd"] = xnd

        def a2():
            s = state[b]
            xnd = s["xnd"]
            xnB, dB = [], []
            for cc in range(CC):
                t_ = xb_p.tile([128, T + 32], BF16, tag=f"xnB_{cc}")
                nc.sync.dma_start_transpose(t_[:],
                                            xnd[:, cc * 128:(cc + 1) * 128])
                xnB.append(t_)
                d = xb_p.tile([128, T], BF16, tag=f"d_{cc}")
                nc.vector.tensor_tensor(d[:], t_[:, 32:T + 32],
                                        t_[:, 31:T + 31], op=OP.subtract)
                dB.append(d)
            s["xnB"], s["dB"] = xnB, dB

        def a3():
            s = state[b]
            xnB, dB = s["xnB"], s["dB"]
            s["xk8"] = mix8(xnB, dB, tmk_c, "k", b, dt=BF16)
            s["xv8"] = mix8(xnB, dB, tmv_c, "v", b, dt=BF16)
            s["xr8"] = mix8(xnB, dB, tmr_c, "r", b)

        return [a00, a0, a1a, a1b, a2, a3]

    def chunks_B(b):
        def mk_hh(hh):
            def f():
                s = state[b]
                xk8, xv8, xr8 = s["xk8"], s["xv8"], s["xr8"]
                if "srw8" not in s:
                    s["srw8"] = srw_p.tile([128, CC, T], FP8, tag="srw8",
                                           name=f"srw_{b}")
                srw8 = s["srw8"]
                e = wk_p.tile([128, T], BF16, tag="e")
                th_t = wk_p.tile([128, T], BF16, tag="tht")
                vsb = wk_p.tile([128, T], BF16, tag="vsb")
                ev = wk_p.tile([128, T], BF16, tag="ev")
                for th in range(2):
                    sl = slice(th * 512, (th + 1) * 512)
                    k_ps = ps.tile([128, 512], F32, tag="ps",
                                   name=f"kps{b}_{hh}{th}")
                    bf_group(k_ps, wkb, xk8, hh, th)
                    r_ps = ps.tile([128, 512], F32, tag="ps",
                                   name=f"rps{b}_{hh}{th}")
                    dr_group(r_ps, wr8, xr8, hh, th)
                    v_ps = ps.tile([128, 512], F32, tag="ps",
                                   name=f"vps{b}_{hh}{th}")
                    bf_group(v_ps, wvb, xv8, hh, th)
                    nc.scalar.activation(e[:, sl], k_ps[:], AF.Exp)
                    nc.scalar.activation(th_t[:, sl], r_ps[:], AF.Tanh,
                                         scale=0.5 / WS)
                    nc.scalar.activation(vsb[:, sl], v_ps[:], AF.Identity)
                nc.vector.tensor_tensor(ev[:], e[:], vsb[:], op=OP.mult)
                if False:
                    pass
                Pb = sc_p.tile([128, T + 1], F32, tag="Pb")
                Qb = sc_p.tile([128, T + 1], F32, tag="Qb")
                nc.vector.memset(Pb[:, 0:1], 0.0)
                nc.vector.memset(Qb[:, 0:1], 0.0)
                db = delta_c[hh][:].to_broadcast((128, T))
                nc.vector.tensor_tensor_scan(Pb[:, 1:T + 1], db, ev[:],
                                             0.0, op0=OP.mult, op1=OP.add)
                nc.vector.tensor_tensor_scan(Qb[:, 1:T + 1], db, e[:],
                                             0.0, op0=OP.mult, op1=OP.add)
                Nt = wt_p.tile([128, T], BF16, tag="Nt")
                Dt = wt_p.tile([128, T], F32, tag="Dt")
                nc.vector.scalar_tensor_tensor(Nt[:], ev[:], eu_c[hh][:],
                                               Pb[:, 0:T], op0=OP.mult,
                                               op1=OP.add)
                nc.vector.scalar_tensor_tensor(Dt[:], e[:], eu_c[hh][:],
                                               Qb[:, 0:T], op0=OP.mult,
                                               op1=OP.add)
                nc.vector.reciprocal_approx_fast(Dt[:], Dt[:])
                yp = wt_p.tile([128, T], BF16, tag="yp")
                nc.vector.tensor_tensor(yp[:], Nt[:], Dt[:], op=OP.mult)
                yth = wt_p.tile([128, T], BF16, tag="yth")
                nc.vector.tensor_tensor(yth[:], yp[:], th_t[:], op=OP.mult)
                nc.vector.tensor_tensor(srw8[:, hh, :], yp[:], yth[:],
                                        op=OP.add)
            return f

        def wo():
            s = state[b]
            xa, srw8 = s["xa"], s["srw8"]
            stats6 = st_p.tile([128, NT, 6], F32, tag="stats2")
            s["stats62"] = stats6
            for n in range(NT):
                p_ = ps.tile([128, 512], F32, tag="ps", name=f"wops{b}_{n}")
                for s_ in range(CC // 2):
                    nc.tensor.matmul(
                        p_[:],
                        srw8[:, 2 * s_:2 * s_ + 2, n * 128:(n + 1) * 128],
                        wo8[:, 2 * s_:2 * s_ + 2, :],
                        start=(s_ == 0), stop=(s_ == CC // 2 - 1),
                        perf_mode=PM.DoubleRow)
                nc.vector.scalar_tensor_tensor(xa[:, n, :], p_[:], 1.0 / WS,
                                               xa[:, n, :], op0=OP.mult,
                                               op1=OP.add)
                nc.vector.bn_stats(stats6[:, n, :], xa[:, n, :])

        return [mk_hh(0), mk_hh(1), mk_hh(2), mk_hh(3), wo]

    def chunks_C(b):
        def c0():
            s = state[b]
            xa = s["xa"]
            rsq_i, mbneg = ln_finish(s["stats62"], "2")
            rstd = rsq_i[:].bitcast(F32)
            xn = xn_p.tile([128, NT, C], BF16, tag="xn_2")
            for n in range(NT):
                nc.scalar.activation(xn[:, n, :], xa[:, n, :], AF.Identity,
                                     bias=mbneg[:, n:n + 1],
                                     scale=rstd[:, n:n + 1])
            xnd = dramp.tile([T + 32, C], BF16, tag="xnd_2")
            nc.sync.dma_start(xnd[0:32, :], zrow[:])
            nc.sync.dma_start(
                xnd[32:T + 32].rearrange("(n p) c -> p n c", p=128), xn[:])
            s["xnd2"] = xnd

        def c1():
            s = state[b]
            xnd = s["xnd2"]
            xnB, dB = [], []
            for cc in range(CC):
                t_ = xb_p.tile([128, T + 32], BF16, tag=f"xnB_{cc}")
                nc.sync.dma_start_transpose(t_[:],
                                            xnd[:, cc * 128:(cc + 1) * 128])
                xnB.append(t_)
                d = xb_p.tile([128, T], BF16, tag=f"d_{cc}")
                nc.vector.tensor_tensor(d[:], t_[:, 32:T + 32],
                                        t_[:, 31:T + 31], op=OP.subtract)
                dB.append(d)
            xk28 = mx_p.tile([128, CC, T], FP8, tag="mx_k2")
            xr28 = mx_p.tile([128, CC, T], FP8, tag="mx_r2")
            for th in range(2):
                mix8_half(xk28, xnB, dB, cmk_c, th)
                mix8_half(xr28, xnB, dB, cmr_c, th)
            s["xk28"], s["xr28"] = xk28, xr28

        return [c0, c1]

    def chunks_D(b):
        def mk_k(th, part):
            def f():
                s = state[b]
                xk28 = s["xk28"]
                if f"kk8_{th}" not in s:
                    s[f"kk8_{th}"] = kk_p.tile([128, HC, 512], FP8,
                                               tag="kk8",
                                               name=f"kk8_{b}_{th}")
                kk8 = s[f"kk8_{th}"]
                for hh in range(part * 4, part * 4 + 4):
                    p_ = ps.tile([128, 512], F32, tag="ps",
                                 name=f"ck{b}_{th}{hh}")
                    dr_group(p_, cwk8, xk28, hh, th)
                    kkb = kk_p.tile([128, 512], BF16, tag="kkb")
                    nc.scalar.activation(kkb[:], p_[:], AF.Relu,
                                         scale=1.0 / WS)
                    nc.scalar.activation(kk8[:, hh, :], kkb[:], AF.Square,
                                         scale=1.0 / 8.0)
            return f

        def mk_rv(th):
            def f():
                s = state[b]
                xa, xr28 = s["xa"], s["xr28"]
                kk8 = s[f"kk8_{th}"]
                yb = y_d[b].rearrange("(n p) c -> p n c", p=128)
                for nn in range(4):
                    n = th * 4 + nn
                    rp = ps.tile([128, 512], F32, tag="ps", name=f"cr{b}_{n}")
                    for s_ in range(CC // 2):
                        nc.tensor.matmul(
                            rp[:],
                            xr28[:, 2 * s_:2 * s_ + 2,
                                 n * 128:(n + 1) * 128],
                            cwr8[:, 2 * s_:2 * s_ + 2, :],
                            start=(s_ == 0), stop=(s_ == CC // 2 - 1),
                            perf_mode=PM.DoubleRow)
                    sig2 = out_p.tile([128, 512], BF16, tag="sig2")
                    nc.scalar.activation(sig2[:], rp[:], AF.Sigmoid,
                                         scale=1.0 / WS)
                    kvp = ps.tile([128, 512], F32, tag="ps", name=f"cv{b}_{n}")
                    for s_ in range(HC // 2):
                        nc.tensor.matmul(
                            kvp[:],
                            kk8[:, 2 * s_:2 * s_ + 2,
                                nn * 128:(nn + 1) * 128],
                            cwv8[:, 2 * s_:2 * s_ + 2, :],
                            start=(s_ == 0), stop=(s_ == HC // 2 - 1),
                            perf_mode=PM.DoubleRow)
                    t2 = out_p.tile([128, 512], BF16, tag="t2")
                    nc.vector.tensor_tensor(t2[:], kvp[:], sig2[:],
                                            op=OP.mult)
                    yo = out_p.tile([128, 512], F32, tag="yo")
                    nc.gpsimd.tensor_tensor(yo[:], t2[:], xa[:, n, :],
                                            op=OP.add)
                    nc.sync.dma_start(yb[:, n, :], yo[:])
            return f

        return [mk_k(0, 0), mk_k(0, 1), mk_k(0, 2), mk_k(0, 3), mk_rv(0),
                mk_k(1, 0), mk_k(1, 1), mk_k(1, 2), mk_k(1, 3), mk_rv(1)]

    # software-pipelined, chunk-interleaved emission:
    #   cycle b: round-robin over B(b), D(b-1), A(b+1)[a0..a2];
    #   then A(b+1).a3 (mixes), then C(b).
    from itertools import zip_longest
    for ch in chunks_A(0):
        ch()
    prev_D = None
    for b in range(bl):
        Bc = chunks_B(b)
        Dc = prev_D or []
        Ac_all = chunks_A(b + 1) if b + 1 < bl else []
        if Ac_all:
            Ac_all[0]()  # xa load for b+1, two cycles ahead of its D
        Ac, Amix = Ac_all[1:5], Ac_all[5:]
        for group in zip_longest(Bc, Dc, Ac):
            for ch in group:
                if ch is not None:
                    ch()
        for ch in Amix:
            ch()
        for ch in chunks_C(b):
            ch()
        prev_D = chunks_D(b)
    for ch in prev_D:
        ch()


def build_program(bl=BL):
    nc = bacc.Bacc("TRN2", target_bir_lowering=False, debug=False,
                   num_devices=NCORES)
    io = {}
    io["x"] = nc.dram_tensor("x", [bl, T, C], F32, kind="ExternalInput")
    io["y"] = nc.dram_tensor("y", [bl, T, C], F32, kind="ExternalOutput")
    for nm, shp in [("wr8", [128, CC, C]), ("wo8", [128, CC, C]),
                    ("cwk8", [128, CC, H]), ("cwr8", [128, CC, C]),
                    ("cwv8", [128, HC, C])]:
        io[nm] = nc.dram_tensor(nm, shp, FP8, kind="ExternalInput")
    for nm in ["wkb", "wvb"]:
        io[nm] = nc.dram_tensor(nm, [128, CC, C], BF16, kind="ExternalInput")
    for nm in ["delta", "expu", "tmk", "tmv", "tmr", "cmk", "cmr"]:
        io[nm] = nc.dram_tensor(nm, [C], F32, kind="ExternalInput")
    import os
    if os.environ.get("KDBG"):
        io["dbg_xnB0"] = nc.dram_tensor("dbg_xnB0", [128, T + 32], BF16,
                                        kind="ExternalOutput")
        io["dbg_xk8"] = nc.dram_tensor("dbg_xk8", [128, CC, T], BF16,
                                       kind="ExternalOutput")
        for nm in ["dbg_e", "dbg_tht", "dbg_ev", "dbg_Nt", "dbg_yp"]:
            io[nm] = nc.dram_tensor(nm, [128, T], BF16 if nm != "dbg_ev" else BF16,
                                    kind="ExternalOutput")
        io["dbg_Pb"] = nc.dram_tensor("dbg_Pb", [128, T + 1], F32,
                                      kind="ExternalOutput")
        io["dbg_x1"] = nc.dram_tensor("dbg_x1", [128, NT, C], F32,
                                      kind="ExternalOutput")
        io["dbg_srw8"] = nc.dram_tensor("dbg_srw8", [128, CC, T], FP8,
                                        kind="ExternalOutput")

    with tile.TileContext(nc) as tc:
        with ExitStack() as ctx:
            _emit(nc, tc, ctx, io, bl)
    nc.compile()
    return nc


def host_params(inputs):
    f32 = np.float32
    g1 = np.asarray(inputs["ln1_g"], f32)
    b1 = np.asarray(inputs["ln1_b"], f32)
    g2 = np.asarray(inputs["ln2_g"], f32)
    b2 = np.asarray(inputs["ln2_b"], f32)
    assert np.allclose(b1, 0.0) and np.allclose(b2, 0.0), \
        "nonzero LN bias not supported"
    Wk = np.asarray(inputs["Wk"], f32)
    Wv = np.asarray(inputs["Wv"], f32)
    Wr = np.asarray(inputs["Wr"], f32)
    Wo = np.asarray(inputs["Wo"], f32)
    cWk = np.asarray(inputs["cWk"], f32)
    cWr = np.asarray(inputs["cWr"], f32)
    cWv = np.asarray(inputs["cWv"], f32)

    fp8 = ml_dtypes.float8_e4m3

    def dr8(WT, scale):
        # WT [K, M] -> [128, K//128, M], k = j*128 + p
        K = WT.shape[0]
        return np.ascontiguousarray(
            (WT * scale).reshape(K // 128, 128, -1).transpose(1, 0, 2)
        ).astype(fp8)

    bfd = ml_dtypes.bfloat16

    def drb(WT):
        K = WT.shape[0]
        return np.ascontiguousarray(
            WT.reshape(K // 128, 128, -1).transpose(1, 0, 2)).astype(bfd)

    p = {
        "wkb": drb(Wk.T * g1[:, None]),
        "wvb": drb(Wv.T * g1[:, None]),
        "wr8": dr8(Wr.T * g1[:, None], WS),
        "wo8": dr8(Wo.T, WS * 0.5),
        "cwk8": dr8(cWk.T * g2[:, None], WS),
        "cwr8": dr8(cWr.T * g2[:, None], WS),
        "cwv8": dr8(cWv.T, WS),
        "delta": np.exp(-np.exp(np.asarray(inputs["time_decay"], f32))),
        "expu": np.exp(np.asarray(inputs["time_first"], f32)),
        "tmk": np.asarray(inputs["tm_k"], f32),
        "tmv": np.asarray(inputs["tm_v"], f32),
        "tmr": np.asarray(inputs["tm_r"], f32),
        "cmk": np.asarray(inputs["cm_k"], f32),
        "cmr": np.asarray(inputs["cm_r"], f32),
    }
    return p


WS = 64.0
_CACHE = {}


def kernel(**inputs):
    from concourse.bass_utils import run_bass_kernel_spmd

    if "nc" not in _CACHE:
        _CACHE["nc"] = build_program(BL)
    nc = _CACHE["nc"]

    p = host_params(inputs)
    x = np.asarray(inputs["x"], np.float32)
    in_maps = []
    for c in range(NCORES):
        m = dict(p)
        m["x"] = np.ascontiguousarray(x[c * BL:(c + 1) * BL])
        in_maps.append(m)
    res = run_bass_kernel_spmd(nc, in_maps, list(range(NCORES)))
    out = np.concatenate([res.results[c]["y"] for c in range(NCORES)], axis=0)
    return out.astype(np.float32)


# revision 4
# speedup vs baseline: 1.0816x; 1.0049x over previous
"""RWKV-v4 block on 8 Trainium2 NeuronCores, data-parallel over batch.

v2: fp8e4 DoubleRow matmuls (2x PE throughput: 256-deep contraction per
512-row instruction), WKV without max-subtraction (numerically safe at
these scales; the M-shift cancels in the P/Q ratio), sigmoid via tanh so
every ACT function in the LN/WKV phase lives in one activation table,
LN rstd via bitcast-Newton rsqrt on DVE (no sqrt table load), and LN2 row
sums ride free on the residual-add's accum_out.

Layouts as v1: layout A [t(128p), n(8), c(512)] for LN stats/residual,
layout B [c(128p) x 4, t(1024)] for mixing/WKV/matmul moving operands,
A->B via bf16 DMA-transpose bounce through DRAM.

Weight scaling: all fp8 weights are pre-scaled by 64 (wo by 32 = 64*0.5
for the tanh-sigmoid halving); the 1/64 compensations fold into ACT
scale arguments and the x1 stt scalar. cWv's 1/64 folds into the
square-ACT scale (1/8)^2.
"""

import numpy as np
import ml_dtypes
from contextlib import ExitStack

import concourse.bass as bass
import concourse.tile as tile
from concourse import bacc, mybir

B, T, C = 32, 1024, 512
H = 4 * C
NCORES = 8
BL = B // NCORES
NT = T // 128
CC = C // 128   # 4 chunks of input/output channels
HC = H // 128   # 16 hidden chunks

F32 = mybir.dt.float32
BF16 = mybir.dt.bfloat16
FP8 = mybir.dt.float8e4
I32 = mybir.dt.int32
AX = mybir.AxisListType
OP = mybir.AluOpType
AF = mybir.ActivationFunctionType
PM = mybir.MatmulPerfMode

WS = 64.0          # weight pre-scale
RSQRT_MAGIC = 0x5f3759df


def _emit(nc, tc, ctx, io, bl):
    x_d = io["x"].ap()
    y_d = io["y"].ap()

    def col(name, c0):
        return io[name].ap()[c0 * 128:(c0 + 1) * 128].rearrange(
            "(c one) -> c one", one=1)

    sb = ctx.enter_context(tc.tile_pool(name="sb", bufs=1))
    ps = ctx.enter_context(tc.tile_pool(name="ps", bufs=8, space="PSUM"))
    dramp = ctx.enter_context(tc.tile_pool(name="dram", bufs=2, space="DRAM"))

    # ---- early x loads (b=0,1) so the pipeline fill isn't behind weights ---
    early_xa = {}
    for b0 in range(min(2, bl)):
        xa_e = None  # placeholder; pools not yet created
    # ---- fp8 weights, DR layout [128, j, m] with k = j*128 + p ----
    def load_w8(name, j, m):
        t_ = sb.tile([128, j, m], FP8, tag=f"w_{name}")
        nc.sync.dma_start(t_[:], io[name].ap())
        return t_

    def load_wb(name, j, m):
        t_ = sb.tile([128, j, m], BF16, tag=f"w_{name}")
        nc.sync.dma_start(t_[:], io[name].ap())
        return t_

    wkb = load_wb("wkb", CC, C)
    wvb = load_wb("wvb", CC, C)
    wr8a = load_w8("wr8a", CC, C)
    wr8b = load_w8("wr8b", CC, C)
    wo8 = load_w8("wo8", CC, C)
    cwk8 = load_w8("cwk8", CC, H)
    cwr8a = load_w8("cwr8a", CC, C)
    cwr8b = load_w8("cwr8b", CC, C)
    cwv8 = load_w8("cwv8", HC, C)

    def vec4(name):
        ts_ = []
        for i in range(CC):
            t_ = sb.tile([128, 1], F32, tag=f"v_{name}_{i}")
            nc.sync.dma_start(t_[:], col(name, i))
            ts_.append(t_)
        return ts_

    delta_c = vec4("delta")
    eu_c = vec4("expu")

    def vec4_m1(name):
        ts_ = []
        for i in range(CC):
            t_ = sb.tile([128, 1], F32, tag=f"vm_{name}_{i}")
            nc.sync.dma_start(t_[:], col(name, i))
            nc.vector.tensor_scalar_add(t_[:], t_[:], -1.0)
            ts_.append(t_)
        return ts_

    tmk_c = vec4_m1("tmk")
    tmv_c = vec4_m1("tmv")
    tmr_c = vec4_m1("tmr")
    cmk_c = vec4_m1("cmk")
    cmr_c = vec4_m1("cmr")

    zrow = sb.tile([32, C], BF16, tag="zrow")
    nc.vector.memset(zrow[:], 0.0)

    # ---- per-batch pools ----
    xa_p = ctx.enter_context(tc.tile_pool(name="xa", bufs=3))
    st_p = ctx.enter_context(tc.tile_pool(name="st", bufs=2))    # stats
    xn_p = ctx.enter_context(tc.tile_pool(name="xn", bufs=1))    # pre-bounce
    xb_p = ctx.enter_context(tc.tile_pool(name="xb", bufs=1))    # layout B
    mx_p = ctx.enter_context(tc.tile_pool(name="mx", bufs=1))    # mixes
    wk_p = ctx.enter_context(tc.tile_pool(name="wkv", bufs=2))   # wkv per-hh
    wt_p = ctx.enter_context(tc.tile_pool(name="wkt", bufs=1))   # wkv tail
    sc_p = ctx.enter_context(tc.tile_pool(name="scan", bufs=1))  # Pb/Qb
    srw_p = ctx.enter_context(tc.tile_pool(name="srw", bufs=1))
    kk_p = ctx.enter_context(tc.tile_pool(name="kk", bufs=1))
    out_p = ctx.enter_context(tc.tile_pool(name="out", bufs=2))

    def rsqrt_dve(vpe, tag):
        """rstd = 1/sqrt(vpe), vpe f32 [128, NT]; bitcast-Newton on DVE."""
        yi = st_p.tile([128, NT], I32, tag=f"rs_i_{tag}")
        nc.vector.tensor_scalar(yi[:], vpe[:].bitcast(I32), 1, None,
                                op0=OP.arith_shift_right)
        nc.vector.tensor_scalar(yi[:], yi[:], -1, RSQRT_MAGIC,
                                op0=OP.mult, op1=OP.add)
        y = yi[:].bitcast(F32)
        a = st_p.tile([128, NT], F32, tag=f"rs_a_{tag}")
        nc.vector.tensor_scalar_mul(a[:], vpe[:], 0.5)
        t1 = st_p.tile([128, NT], F32, tag=f"rs_t_{tag}")
        for _ in range(2):
            nc.vector.tensor_tensor(t1[:], y, y, op=OP.mult)
            nc.vector.tensor_tensor(t1[:], t1[:], a[:], op=OP.mult)
            nc.vector.tensor_scalar(t1[:], t1[:], -1.0, 1.5,
                                    op0=OP.mult, op1=OP.add)
            nc.vector.tensor_tensor(y, y, t1[:], op=OP.mult)
        return yi  # f32 view via bitcast

    def ln_finish_sums(sums, sqs, tag):
        """LN finish from ACT-accumulated sums/sumsq."""
        mean = st_p.tile([128, NT], F32, tag=f"mean_{tag}")
        nc.vector.tensor_scalar_mul(mean[:], sums[:], 1.0 / C)
        var = st_p.tile([128, NT], F32, tag=f"var_{tag}")
        nc.vector.tensor_tensor(var[:], mean[:], mean[:], op=OP.mult)
        nc.vector.scalar_tensor_tensor(var[:], sqs[:], 1.0 / C, var[:],
                                       op0=OP.mult, op1=OP.subtract)
        nc.vector.tensor_scalar_add(var[:], var[:], 1e-5)
        rsq_i = rsqrt_dve(var, tag)
        rstd = rsq_i[:].bitcast(F32)
        mbneg = st_p.tile([128, NT], F32, tag=f"mb_{tag}")
        nc.vector.scalar_tensor_tensor(mbneg[:], mean[:], -1.0, rstd,
                                       op0=OP.mult, op1=OP.mult)
        return rsq_i, mbneg

    def ln_finish(stats6, tag):
        """stats6 [128, NT, 6] from bn_stats (2 groups of count/mean/M2).
        -> (rstd-int tile (bitcast f32), mbneg f32 [128,NT])."""
        mv2 = st_p.tile([128, NT, 2], F32, tag=f"mv_{tag}")
        for n in range(NT):
            nc.vector.bn_aggr(mv2[:, n, :], stats6[:, n, :])
        mv = mv2[:, :, 0:1].rearrange("p n o -> p (n o)")
        var = st_p.tile([128, NT], F32, tag=f"var_{tag}")
        nc.vector.tensor_scalar(var[:],
                                mv2[:, :, 1:2].rearrange("p n o -> p (n o)"),
                                1.0, 1e-5, op0=OP.mult, op1=OP.add)
        rsq_i = rsqrt_dve(var, tag)
        rstd = rsq_i[:].bitcast(F32)
        mbneg = st_p.tile([128, NT], F32, tag=f"mb_{tag}")
        nc.vector.scalar_tensor_tensor(mbneg[:], mv, -1.0, rstd,
                                       op0=OP.mult, op1=OP.mult)
        return rsq_i, mbneg

    def ln_bounce(xa_t, rsq_i, mbneg, which, b):
        """normalize (ACT) -> bounce -> transpose -> delta. Returns
        (xnB list, d list)."""
        rstd = rsq_i[:].bitcast(F32)
        xn = xn_p.tile([128, NT, C], BF16, tag=f"xn_{which}")
        for n in range(NT):
            nc.scalar.activation(xn[:, n, :], xa_t[:, n, :], AF.Identity,
                                 bias=mbneg[:, n:n + 1],
                                 scale=rstd[:, n:n + 1])
        xnd = dramp.tile([T + 32, C], BF16, tag=f"xnd_{which}")
        nc.sync.dma_start(xnd[0:32, :], zrow[:])
        nc.sync.dma_start(xnd[32:T + 32].rearrange("(n p) c -> p n c", p=128),
                          xn[:])
        xnB, dB = [], []
        for cc in range(CC):
            t_ = xb_p.tile([128, T + 32], BF16, tag=f"xnB_{which}_{cc}")
            nc.sync.dma_start_transpose(t_[:],
                                        xnd[:, cc * 128:(cc + 1) * 128])
            xnB.append(t_)
            d = xb_p.tile([128, T], BF16, tag=f"d_{which}_{cc}")
            nc.gpsimd.tensor_tensor(d[:], t_[:, 32:T + 32], t_[:, 31:T + 31],
                                    op=OP.subtract)
            dB.append(d)
        return xnB, dB

    def mix8(xnB, dB, coefm1, tag, b, dt=FP8):
        """mix tile [128, CC, T]: out = xn + (coef-1)*d."""
        m = mx_p.tile([128, CC, T], dt, tag=f"mx_{tag}")
        for cc in range(CC):
            nc.vector.scalar_tensor_tensor(m[:, cc, :], dB[cc][:],
                                           coefm1[cc][:],
                                           xnB[cc][:, 32:T + 32],
                                           op0=OP.mult, op1=OP.add)
        return m

    def mix8_half(m, xnB, dB, coefm1, th):
        """fill th-half of a mix tile: cols th*512..(th+1)*512."""
        sl = slice(th * 512, (th + 1) * 512)
        sl32 = slice(32 + th * 512, 32 + (th + 1) * 512)
        for cc in range(CC):
            nc.vector.scalar_tensor_tensor(m[:, cc, sl], dB[cc][:, sl],
                                           coefm1[cc][:],
                                           xnB[cc][:, sl32],
                                           op0=OP.mult, op1=OP.add)

    def bf_group(out_ps, wb_, mb_, hh, th):
        for ci in range(CC):
            nc.tensor.matmul(
                out_ps[:],
                wb_[:, ci, hh * 128:(hh + 1) * 128],
                mb_[:, ci, th * 512:(th + 1) * 512],
                start=(ci == 0), stop=(ci == CC - 1))

    def dr_group(out_ps, w8, m8, hh, th, nsteps=CC // 2):
        """Accumulate DR matmuls: out += w8[:,2s:2s+2, hh*128:+128].T @
        m8[:,2s:2s+2, th*512:+512] over s."""
        for s in range(nsteps):
            nc.tensor.matmul(
                out_ps[:],
                w8[:, 2 * s:2 * s + 2, hh * 128:(hh + 1) * 128],
                m8[:, 2 * s:2 * s + 2, th * 512:(th + 1) * 512],
                start=(s == 0), stop=(s == nsteps - 1),
                perf_mode=PM.DoubleRow)

    # ================= per-batch stages (chunked) =================
    state = {}

    def chunks_A(b):
        def a00():
            xb = x_d[b].rearrange("(n p) c -> p n c", p=128)
            xa = xa_p.tile([128, NT, C], F32, tag="xa", name=f"xa_{b}")
            nc.sync.dma_start(xa[:], xb)
            state[b] = dict(xa=xa)

        def a0():
            xa = state[b]["xa"]
            stats6 = st_p.tile([128, NT, 6], F32, tag="stats1")
            for n in range(NT):
                nc.vector.bn_stats(stats6[:, n, :], xa[:, n, :])
            state[b]["stats6"] = stats6

        def a1a():
            s = state[b]
            rsq_i, mbneg = ln_finish(s["stats6"], "1")
            s["rsq_i"], s["mbneg"] = rsq_i, mbneg
            xn = xn_p.tile([128, NT, C], BF16, tag="xn_1")
            s["xn"] = xn
            rstd = rsq_i[:].bitcast(F32)
            for n in range(NT // 2):
                nc.scalar.activation(xn[:, n, :], s["xa"][:, n, :],
                                     AF.Identity, bias=mbneg[:, n:n + 1],
                                     scale=rstd[:, n:n + 1])

        def a1b():
            s = state[b]
            xn = s["xn"]
            rstd = s["rsq_i"][:].bitcast(F32)
            mbneg = s["mbneg"]
            for n in range(NT // 2, NT):
                nc.scalar.activation(xn[:, n, :], s["xa"][:, n, :],
                                     AF.Identity, bias=mbneg[:, n:n + 1],
                                     scale=rstd[:, n:n + 1])
            xnd = dramp.tile([T + 32, C], BF16, tag="xnd_1")
            nc.sync.dma_start(xnd[0:32, :], zrow[:])
            nc.sync.dma_start(
                xnd[32:T + 32].rearrange("(n p) c -> p n c", p=128), xn[:])
            s["xnd"] = xnd

        def a2():
            s = state[b]
            xnd = s["xnd"]
            xnB, dB = [], []
            for cc in range(CC):
                t_ = xb_p.tile([128, T + 32], BF16, tag=f"xnB_{cc}")
                nc.sync.dma_start_transpose(t_[:],
                                            xnd[:, cc * 128:(cc + 1) * 128])
                xnB.append(t_)
                d = xb_p.tile([128, T], BF16, tag=f"d_{cc}")
                nc.vector.tensor_tensor(d[:], t_[:, 32:T + 32],
                                        t_[:, 31:T + 31], op=OP.subtract)
                dB.append(d)
            xn8 = mx_p.tile([128, CC, T + 32], FP8, tag="xn8")
            for cc in range(CC):
                nc.scalar.activation(xn8[:, cc, :], xnB[cc][:], AF.Identity)
            s["xn8"] = xn8
            s["xnB"], s["dB"] = xnB, dB

        def a3():
            s = state[b]
            xnB, dB = s["xnB"], s["dB"]
            s["xk8"] = mix8(xnB, dB, tmk_c, "k", b, dt=BF16)
            s["xv8"] = mix8(xnB, dB, tmv_c, "v", b, dt=BF16)

        return [a00, a0, a1a, a1b, a2, a3]

    def chunks_B(b):
        def mk_hh(hh):
            def f():
                s = state[b]
                xk8, xv8, xn8 = s["xk8"], s["xv8"], s["xn8"]
                if "srw8" not in s:
                    s["srw8"] = srw_p.tile([128, CC, T], FP8, tag="srw8",
                                           name=f"srw_{b}")
                srw8 = s["srw8"]
                e = wk_p.tile([128, T], BF16, tag="e")
                th_t = wk_p.tile([128, T], BF16, tag="tht")
                vsb = wk_p.tile([128, T], BF16, tag="vsb")
                ev = wk_p.tile([128, T], BF16, tag="ev")
                for th in range(2):
                    sl = slice(th * 512, (th + 1) * 512)
                    k_ps = ps.tile([128, 512], F32, tag="ps",
                                   name=f"kps{b}_{hh}{th}")
                    bf_group(k_ps, wkb, xk8, hh, th)
                    r_ps = ps.tile([128, 512], F32, tag="ps",
                                   name=f"rps{b}_{hh}{th}")
                    for s_ in range(CC // 2):
                        nc.tensor.matmul(
                            r_ps[:],
                            wr8a[:, 2 * s_:2 * s_ + 2,
                                 hh * 128:(hh + 1) * 128],
                            xn8[:, 2 * s_:2 * s_ + 2,
                                32 + th * 512:32 + (th + 1) * 512],
                            start=(s_ == 0), stop=False,
                            perf_mode=PM.DoubleRow)
                    for s_ in range(CC // 2):
                        nc.tensor.matmul(
                            r_ps[:],
                            wr8b[:, 2 * s_:2 * s_ + 2,
                                 hh * 128:(hh + 1) * 128],
                            xn8[:, 2 * s_:2 * s_ + 2,
                                31 + th * 512:31 + (th + 1) * 512],
                            start=False, stop=(s_ == CC // 2 - 1),
                            perf_mode=PM.DoubleRow)
                    v_ps = ps.tile([128, 512], F32, tag="ps",
                                   name=f"vps{b}_{hh}{th}")
                    bf_group(v_ps, wvb, xv8, hh, th)
                    nc.scalar.activation(e[:, sl], k_ps[:], AF.Exp)
                    nc.scalar.activation(th_t[:, sl], r_ps[:], AF.Tanh,
                                         scale=0.5 / WS)
                    nc.scalar.activation(vsb[:, sl], v_ps[:], AF.Identity)
                nc.vector.tensor_tensor(ev[:], e[:], vsb[:], op=OP.mult)
                if False:
                    pass
                Pb = sc_p.tile([128, T + 1], F32, tag="Pb")
                Qb = sc_p.tile([128, T + 1], F32, tag="Qb")
                nc.vector.memset(Pb[:, 0:1], 0.0)
                nc.vector.memset(Qb[:, 0:1], 0.0)
                db = delta_c[hh][:].to_broadcast((128, T))
                nc.vector.tensor_tensor_scan(Pb[:, 1:T + 1], db, ev[:],
                                             0.0, op0=OP.mult, op1=OP.add)
                nc.vector.tensor_tensor_scan(Qb[:, 1:T + 1], db, e[:],
                                             0.0, op0=OP.mult, op1=OP.add)
                Nt = wt_p.tile([128, T], BF16, tag="Nt")
                Dt = wt_p.tile([128, T], F32, tag="Dt")
                nc.vector.scalar_tensor_tensor(Nt[:], ev[:], eu_c[hh][:],
                                               Pb[:, 0:T], op0=OP.mult,
                                               op1=OP.add)
                nc.vector.scalar_tensor_tensor(Dt[:], e[:], eu_c[hh][:],
                                               Qb[:, 0:T], op0=OP.mult,
                                               op1=OP.add)
                nc.vector.reciprocal_approx_fast(Dt[:], Dt[:])
                yp = wt_p.tile([128, T], BF16, tag="yp")
                nc.vector.tensor_tensor(yp[:], Nt[:], Dt[:], op=OP.mult)
                yth = wt_p.tile([128, T], BF16, tag="yth")
                nc.vector.tensor_tensor(yth[:], yp[:], th_t[:], op=OP.mult)
                nc.vector.tensor_tensor(srw8[:, hh, :], yp[:], yth[:],
                                        op=OP.add)
            return f

        def wo():
            s = state[b]
            xa, srw8 = s["xa"], s["srw8"]
            stats6 = st_p.tile([128, NT, 6], F32, tag="stats2")
            s["stats62"] = stats6
            for n in range(NT):
                p_ = ps.tile([128, 512], F32, tag="ps", name=f"wops{b}_{n}")
                for s_ in range(CC // 2):
                    nc.tensor.matmul(
                        p_[:],
                        srw8[:, 2 * s_:2 * s_ + 2, n * 128:(n + 1) * 128],
                        wo8[:, 2 * s_:2 * s_ + 2, :],
                        start=(s_ == 0), stop=(s_ == CC // 2 - 1),
                        perf_mode=PM.DoubleRow)
                nc.vector.scalar_tensor_tensor(xa[:, n, :], p_[:], 1.0 / WS,
                                               xa[:, n, :], op0=OP.mult,
                                               op1=OP.add)
                nc.vector.bn_stats(stats6[:, n, :], xa[:, n, :])

        return [mk_hh(0), mk_hh(1), mk_hh(2), mk_hh(3), wo]

    def chunks_C(b):
        def c0a():
            s = state[b]
            xa = s["xa"]
            rsq_i, mbneg = ln_finish(s["stats62"], "2")
            rstd = rsq_i[:].bitcast(F32)
            s["rsq2"], s["mb2"] = rsq_i, mbneg
            xn = xn_p.tile([128, NT, C], BF16, tag="xn_2")
            s["xn2"] = xn
            xnd = dramp.tile([T + 32, C], BF16, tag="xnd_2")
            s["xnd2"] = xnd
            for n in range(NT // 2):
                nc.scalar.activation(xn[:, n, :], xa[:, n, :], AF.Identity,
                                     bias=mbneg[:, n:n + 1],
                                     scale=rstd[:, n:n + 1])
            nc.sync.dma_start(xnd[0:32, :], zrow[:])
            nc.sync.dma_start(
                xnd[32:32 + 512].rearrange("(n p) c -> p n c", p=128),
                xn[:, 0:NT // 2, :])

        def c0b():
            s = state[b]
            xa, xn, xnd = s["xa"], s["xn2"], s["xnd2"]
            rstd = s["rsq2"][:].bitcast(F32)
            mbneg = s["mb2"]
            for n in range(NT // 2, NT):
                nc.scalar.activation(xn[:, n, :], xa[:, n, :], AF.Identity,
                                     bias=mbneg[:, n:n + 1],
                                     scale=rstd[:, n:n + 1])
            nc.sync.dma_start(
                xnd[32 + 512:T + 32].rearrange("(n p) c -> p n c", p=128),
                xn[:, NT // 2:NT, :])

        def c1a():
            s = state[b]
            xnd = s["xnd2"]
            xnB, dB = [], []
            for cc in range(CC):
                t_ = xb_p.tile([128, T + 32], BF16, tag=f"xnB_{cc}")
                nc.sync.dma_start_transpose(
                    t_[:, 0:544], xnd[0:544, cc * 128:(cc + 1) * 128])
                xnB.append(t_)
                d = xb_p.tile([128, T], BF16, tag=f"d_{cc}")
                nc.vector.tensor_tensor(d[:, 0:512], t_[:, 32:544],
                                        t_[:, 31:543], op=OP.subtract)
                dB.append(d)
            s["xnB2"], s["dB2"] = xnB, dB
            xn28 = mx_p.tile([128, CC, T + 32], FP8, tag="xn28")
            s["xn28"] = xn28
            for cc in range(CC):
                nc.scalar.activation(xn28[:, cc, 0:544], xnB[cc][:, 0:544],
                                     AF.Identity)
            xk28 = mx_p.tile([128, CC, T], FP8, tag="mx_k2")
            s["xk28"] = xk28
            mix8_half(xk28, xnB, dB, cmk_c, 0)

        def c1b():
            s = state[b]
            xnd = s["xnd2"]
            xnB, dB = s["xnB2"], s["dB2"]
            for cc in range(CC):
                t_ = xnB[cc]
                nc.sync.dma_start_transpose(
                    t_[:, 544:T + 32],
                    xnd[544:T + 32, cc * 128:(cc + 1) * 128])
                nc.vector.tensor_tensor(dB[cc][:, 512:1024],
                                        t_[:, 544:T + 32],
                                        t_[:, 543:T + 31], op=OP.subtract)
            xn28 = s["xn28"]
            for cc in range(CC):
                nc.scalar.activation(xn28[:, cc, 544:T + 32],
                                     xnB[cc][:, 544:T + 32], AF.Identity)
            mix8_half(s["xk28"], xnB, dB, cmk_c, 1)

        return [c0a, c0b, c1a, c1b]

    def chunks_D(b):
        def mk_k(th, part):
            def f():
                s = state[b]
                xk28 = s["xk28"]
                if f"kk8_{th}" not in s:
                    s[f"kk8_{th}"] = kk_p.tile([128, HC, 512], FP8,
                                               tag="kk8",
                                               name=f"kk8_{b}_{th}")
                kk8 = s[f"kk8_{th}"]
                for hh in range(part * 4, part * 4 + 4):
                    p_ = ps.tile([128, 512], F32, tag="ps",
                                 name=f"ck{b}_{th}{hh}")
                    dr_group(p_, cwk8, xk28, hh, th)
                    kkb = kk_p.tile([128, 512], BF16, tag="kkb")
                    nc.scalar.activation(kkb[:], p_[:], AF.Relu,
                                         scale=1.0 / WS)
                    nc.scalar.activation(kk8[:, hh, :], kkb[:], AF.Square,
                                         scale=1.0 / 8.0)
            return f

        def mk_rv(th):
            def f():
                s = state[b]
                xa, xn28 = s["xa"], s["xn28"]
                kk8 = s[f"kk8_{th}"]
                yb = y_d[b].rearrange("(n p) c -> p n c", p=128)
                for nn in range(4):
                    n = th * 4 + nn
                    rp = ps.tile([128, 512], F32, tag="ps", name=f"cr{b}_{n}")
                    for s_ in range(CC // 2):
                        nc.tensor.matmul(
                            rp[:],
                            xn28[:, 2 * s_:2 * s_ + 2,
                                 32 + n * 128:32 + (n + 1) * 128],
                            cwr8a[:, 2 * s_:2 * s_ + 2, :],
                            start=(s_ == 0), stop=False,
                            perf_mode=PM.DoubleRow)
                    for s_ in range(CC // 2):
                        nc.tensor.matmul(
                            rp[:],
                            xn28[:, 2 * s_:2 * s_ + 2,
                                 31 + n * 128:31 + (n + 1) * 128],
                            cwr8b[:, 2 * s_:2 * s_ + 2, :],
                            start=False, stop=(s_ == CC // 2 - 1),
                            perf_mode=PM.DoubleRow)
                    sig2 = out_p.tile([128, 512], BF16, tag="sig2")
                    nc.scalar.activation(sig2[:], rp[:], AF.Sigmoid,
                                         scale=1.0 / WS)
                    kvp = ps.tile([128, 512], F32, tag="ps", name=f"cv{b}_{n}")
                    for s_ in range(HC // 2):
                        nc.tensor.matmul(
                            kvp[:],
                            kk8[:, 2 * s_:2 * s_ + 2,
                                nn * 128:(nn + 1) * 128],
                            cwv8[:, 2 * s_:2 * s_ + 2, :],
                            start=(s_ == 0), stop=(s_ == HC // 2 - 1),
                            perf_mode=PM.DoubleRow)
                    t2 = out_p.tile([128, 512], BF16, tag="t2")
                    nc.vector.tensor_tensor(t2[:], kvp[:], sig2[:],
                                            op=OP.mult)
                    yo = out_p.tile([128, 512], F32, tag="yo")
                    nc.gpsimd.tensor_tensor(yo[:], t2[:], xa[:, n, :],
                                            op=OP.add)
                    nc.sync.dma_start(yb[:, n, :], yo[:])
            return f

        return [mk_k(0, 0), mk_k(0, 1), mk_k(0, 2), mk_k(0, 3), mk_rv(0),
                mk_k(1, 0), mk_k(1, 1), mk_k(1, 2), mk_k(1, 3), mk_rv(1)]

    # software-pipelined, chunk-interleaved emission:
    #   cycle b: round-robin over B(b), D(b-1), A(b+1)[a0..a2];
    #   then A(b+1).a3 (mixes), then C(b).
    from itertools import zip_longest
    for ch in chunks_A(0):
        ch()
    prev_D = None
    for b in range(bl):
        Bc = chunks_B(b)
        Dc = prev_D or []
        Ac_all = chunks_A(b + 1) if b + 1 < bl else []
        if Ac_all:
            Ac_all[0]()  # xa load for b+1, two cycles ahead of its D
        Ac, Amix = Ac_all[1:5], Ac_all[5:]
        for group in zip_longest(Bc, Dc, Ac):
            for ch in group:
                if ch is not None:
                    ch()
        for ch in Amix:
            ch()
        for ch in chunks_C(b):
            ch()
        prev_D = chunks_D(b)
    for ch in prev_D:
        ch()


def build_program(bl=BL):
    nc = bacc.Bacc("TRN2", target_bir_lowering=False, debug=False,
                   num_devices=NCORES)
    io = {}
    io["x"] = nc.dram_tensor("x", [bl, T, C], F32, kind="ExternalInput")
    io["y"] = nc.dram_tensor("y", [bl, T, C], F32, kind="ExternalOutput")
    for nm, shp in [("wr8a", [128, CC, C]), ("wr8b", [128, CC, C]),
                    ("wo8", [128, CC, C]),
                    ("cwk8", [128, CC, H]), ("cwr8a", [128, CC, C]),
                    ("cwr8b", [128, CC, C]), ("cwv8", [128, HC, C])]:
        io[nm] = nc.dram_tensor(nm, shp, FP8, kind="ExternalInput")
    for nm in ["wkb", "wvb"]:
        io[nm] = nc.dram_tensor(nm, [128, CC, C], BF16, kind="ExternalInput")
    for nm in ["delta", "expu", "tmk", "tmv", "tmr", "cmk", "cmr"]:
        io[nm] = nc.dram_tensor(nm, [C], F32, kind="ExternalInput")
    import os
    if os.environ.get("KDBG"):
        io["dbg_xnB0"] = nc.dram_tensor("dbg_xnB0", [128, T + 32], BF16,
                                        kind="ExternalOutput")
        io["dbg_xk8"] = nc.dram_tensor("dbg_xk8", [128, CC, T], BF16,
                                       kind="ExternalOutput")
        for nm in ["dbg_e", "dbg_tht", "dbg_ev", "dbg_Nt", "dbg_yp"]:
            io[nm] = nc.dram_tensor(nm, [128, T], BF16 if nm != "dbg_ev" else BF16,
                                    kind="ExternalOutput")
        io["dbg_Pb"] = nc.dram_tensor("dbg_Pb", [128, T + 1], F32,
                                      kind="ExternalOutput")
        io["dbg_x1"] = nc.dram_tensor("dbg_x1", [128, NT, C], F32,
                                      kind="ExternalOutput")
        io["dbg_srw8"] = nc.dram_tensor("dbg_srw8", [128, CC, T], FP8,
                                        kind="ExternalOutput")

    with tile.TileContext(nc) as tc:
        with ExitStack() as ctx:
            _emit(nc, tc, ctx, io, bl)
    nc.compile()
    return nc


def host_params(inputs):
    f32 = np.float32
    g1 = np.asarray(inputs["ln1_g"], f32)
    b1 = np.asarray(inputs["ln1_b"], f32)
    g2 = np.asarray(inputs["ln2_g"], f32)
    b2 = np.asarray(inputs["ln2_b"], f32)
    assert np.allclose(b1, 0.0) and np.allclose(b2, 0.0), \
        "nonzero LN bias not supported"
    Wk = np.asarray(inputs["Wk"], f32)
    Wv = np.asarray(inputs["Wv"], f32)
    Wr = np.asarray(inputs["Wr"], f32)
    Wo = np.asarray(inputs["Wo"], f32)
    cWk = np.asarray(inputs["cWk"], f32)
    cWr = np.asarray(inputs["cWr"], f32)
    cWv = np.asarray(inputs["cWv"], f32)

    fp8 = ml_dtypes.float8_e4m3

    def dr8(WT, scale):
        # WT [K, M] -> [128, K//128, M], k = j*128 + p
        K = WT.shape[0]
        return np.ascontiguousarray(
            (WT * scale).reshape(K // 128, 128, -1).transpose(1, 0, 2)
        ).astype(fp8)

    bfd = ml_dtypes.bfloat16

    def drb(WT):
        K = WT.shape[0]
        return np.ascontiguousarray(
            WT.reshape(K // 128, 128, -1).transpose(1, 0, 2)).astype(bfd)

    p = {
        "wkb": drb(Wk.T * g1[:, None]),
        "wvb": drb(Wv.T * g1[:, None]),
        "wr8a": dr8(Wr.T * (g1 * np.asarray(inputs["tm_r"], f32))[:, None],
                    WS),
        "wr8b": dr8(Wr.T * (g1 * (1.0 - np.asarray(inputs["tm_r"],
                                                   f32)))[:, None], WS),
        "wo8": dr8(Wo.T, WS * 0.5),
        "cwk8": dr8(cWk.T * g2[:, None], WS),
        "cwr8a": dr8(cWr.T * (g2 * np.asarray(inputs["cm_r"], f32))[:, None],
                     WS),
        "cwr8b": dr8(cWr.T * (g2 * (1.0 - np.asarray(inputs["cm_r"],
                                                     f32)))[:, None], WS),
        "cwv8": dr8(cWv.T, WS),
        "delta": np.exp(-np.exp(np.asarray(inputs["time_decay"], f32))),
        "expu": np.exp(np.asarray(inputs["time_first"], f32)),
        "tmk": np.asarray(inputs["tm_k"], f32),
        "tmv": np.asarray(inputs["tm_v"], f32),
        "tmr": np.asarray(inputs["tm_r"], f32),
        "cmk": np.asarray(inputs["cm_k"], f32),
        "cmr": np.asarray(inputs["cm_r"], f32),
    }
    return p


WS = 64.0
_CACHE = {}


def kernel(**inputs):
    from concourse.bass_utils import run_bass_kernel_spmd

    if "nc" not in _CACHE:
        _CACHE["nc"] = build_program(BL)
    nc = _CACHE["nc"]

    p = host_params(inputs)
    x = np.asarray(inputs["x"], np.float32)
    in_maps = []
    for c in range(NCORES):
        m = dict(p)
        m["x"] = np.ascontiguousarray(x[c * BL:(c + 1) * BL])
        in_maps.append(m)
    res = run_bass_kernel_spmd(nc, in_maps, list(range(NCORES)))
    out = np.concatenate([res.results[c]["y"] for c in range(NCORES)], axis=0)
    return out.astype(np.float32)


# revision 5
# speedup vs baseline: 1.1258x; 1.0409x over previous
"""RWKV-v4 block on 8 Trainium2 NeuronCores, data-parallel over batch.

v2: fp8e4 DoubleRow matmuls (2x PE throughput: 256-deep contraction per
512-row instruction), WKV without max-subtraction (numerically safe at
these scales; the M-shift cancels in the P/Q ratio), sigmoid via tanh so
every ACT function in the LN/WKV phase lives in one activation table,
LN rstd via bitcast-Newton rsqrt on DVE (no sqrt table load), and LN2 row
sums ride free on the residual-add's accum_out.

Layouts as v1: layout A [t(128p), n(8), c(512)] for LN stats/residual,
layout B [c(128p) x 4, t(1024)] for mixing/WKV/matmul moving operands,
A->B via bf16 DMA-transpose bounce through DRAM.

Weight scaling: all fp8 weights are pre-scaled by 64 (wo by 32 = 64*0.5
for the tanh-sigmoid halving); the 1/64 compensations fold into ACT
scale arguments and the x1 stt scalar. cWv's 1/64 folds into the
square-ACT scale (1/8)^2.
"""

import numpy as np
import ml_dtypes
from contextlib import ExitStack

import concourse.bass as bass
import concourse.tile as tile
from concourse import bacc, mybir

B, T, C = 32, 1024, 512
H = 4 * C
NCORES = 8
BL = B // NCORES
NT = T // 128
CC = C // 128   # 4 chunks of input/output channels
HC = H // 128   # 16 hidden chunks

F32 = mybir.dt.float32
BF16 = mybir.dt.bfloat16
FP8 = mybir.dt.float8e4
I32 = mybir.dt.int32
AX = mybir.AxisListType
OP = mybir.AluOpType
AF = mybir.ActivationFunctionType
PM = mybir.MatmulPerfMode

WS = 64.0          # weight pre-scale
RSQRT_MAGIC = 0x5f3759df


def _emit(nc, tc, ctx, io, bl):
    x_d = io["x"].ap()
    y_d = io["y"].ap()

    def col(name, c0):
        return io[name].ap()[c0 * 128:(c0 + 1) * 128].rearrange(
            "(c one) -> c one", one=1)

    sb = ctx.enter_context(tc.tile_pool(name="sb", bufs=1))
    ps = ctx.enter_context(tc.tile_pool(name="ps", bufs=8, space="PSUM"))
    dramp = ctx.enter_context(tc.tile_pool(name="dram", bufs=2, space="DRAM"))

    # ---- early x loads (b=0,1) so the pipeline fill isn't behind weights ---
    xa_p = ctx.enter_context(tc.tile_pool(name="xa", bufs=3))
    state = {}
    for b0 in range(min(2, bl)):
        xa_e = xa_p.tile([128, NT, C], F32, tag="xa", name=f"xa_{b0}")
        nc.sync.dma_start(xa_e[:], x_d[b0].rearrange("(n p) c -> p n c",
                                                     p=128))
        state[b0] = dict(xa=xa_e)
    # ---- fp8 weights, DR layout [128, j, m] with k = j*128 + p ----
    def load_w8(name, j, m):
        t_ = sb.tile([128, j, m], FP8, tag=f"w_{name}")
        nc.sync.dma_start(t_[:], io[name].ap())
        return t_

    def load_wb(name, j, m):
        t_ = sb.tile([128, j, m], BF16, tag=f"w_{name}")
        nc.sync.dma_start(t_[:], io[name].ap())
        return t_

    wkb = load_wb("wkb", CC, C)
    wvb = load_wb("wvb", CC, C)
    wr8a = load_w8("wr8a", CC, C)
    wr8b = load_w8("wr8b", CC, C)
    wo8 = load_w8("wo8", CC, C)
    cwk8 = load_w8("cwk8", CC, H)
    cwr8a = load_w8("cwr8a", CC, C)
    cwr8b = load_w8("cwr8b", CC, C)
    cwv8 = load_w8("cwv8", HC, C)

    def vec4(name):
        ts_ = []
        for i in range(CC):
            t_ = sb.tile([128, 1], F32, tag=f"v_{name}_{i}")
            nc.sync.dma_start(t_[:], col(name, i))
            ts_.append(t_)
        return ts_

    delta_c = vec4("delta")
    eu_c = vec4("expu")

    def vec4_m1(name):
        ts_ = []
        for i in range(CC):
            t_ = sb.tile([128, 1], F32, tag=f"vm_{name}_{i}")
            nc.sync.dma_start(t_[:], col(name, i))
            nc.vector.tensor_scalar_add(t_[:], t_[:], -1.0)
            ts_.append(t_)
        return ts_

    tmk_c = vec4_m1("tmk")
    tmv_c = vec4_m1("tmv")
    tmr_c = vec4_m1("tmr")
    cmk_c = vec4_m1("cmk")
    cmr_c = vec4_m1("cmr")

    zrow = sb.tile([32, C], BF16, tag="zrow")
    nc.vector.memset(zrow[:], 0.0)

    # ---- per-batch pools ----
    st_p = ctx.enter_context(tc.tile_pool(name="st", bufs=2))    # stats
    xn_p = ctx.enter_context(tc.tile_pool(name="xn", bufs=1))    # pre-bounce
    xb_p = ctx.enter_context(tc.tile_pool(name="xb", bufs=1))    # layout B
    mx_p = ctx.enter_context(tc.tile_pool(name="mx", bufs=1))    # mixes
    wk_p = ctx.enter_context(tc.tile_pool(name="wkv", bufs=2))   # wkv per-hh
    wt_p = ctx.enter_context(tc.tile_pool(name="wkt", bufs=1))   # wkv tail
    sc_p = ctx.enter_context(tc.tile_pool(name="scan", bufs=1))  # Pb/Qb
    srw_p = ctx.enter_context(tc.tile_pool(name="srw", bufs=1))
    kk_p = ctx.enter_context(tc.tile_pool(name="kk", bufs=1))
    out_p = ctx.enter_context(tc.tile_pool(name="out", bufs=2))

    def rsqrt_dve(vpe, tag):
        """rstd = 1/sqrt(vpe), vpe f32 [128, NT]; bitcast-Newton on DVE."""
        yi = st_p.tile([128, NT], I32, tag=f"rs_i_{tag}")
        nc.vector.tensor_scalar(yi[:], vpe[:].bitcast(I32), 1, None,
                                op0=OP.arith_shift_right)
        nc.vector.tensor_scalar(yi[:], yi[:], -1, RSQRT_MAGIC,
                                op0=OP.mult, op1=OP.add)
        y = yi[:].bitcast(F32)
        a = st_p.tile([128, NT], F32, tag=f"rs_a_{tag}")
        nc.vector.tensor_scalar_mul(a[:], vpe[:], 0.5)
        t1 = st_p.tile([128, NT], F32, tag=f"rs_t_{tag}")
        for _ in range(2):
            nc.vector.tensor_tensor(t1[:], y, y, op=OP.mult)
            nc.vector.tensor_tensor(t1[:], t1[:], a[:], op=OP.mult)
            nc.vector.tensor_scalar(t1[:], t1[:], -1.0, 1.5,
                                    op0=OP.mult, op1=OP.add)
            nc.vector.tensor_tensor(y, y, t1[:], op=OP.mult)
        return yi  # f32 view via bitcast

    def ln_finish_sums(sums, sqs, tag):
        """LN finish from ACT-accumulated sums/sumsq."""
        mean = st_p.tile([128, NT], F32, tag=f"mean_{tag}")
        nc.vector.tensor_scalar_mul(mean[:], sums[:], 1.0 / C)
        var = st_p.tile([128, NT], F32, tag=f"var_{tag}")
        nc.vector.tensor_tensor(var[:], mean[:], mean[:], op=OP.mult)
        nc.vector.scalar_tensor_tensor(var[:], sqs[:], 1.0 / C, var[:],
                                       op0=OP.mult, op1=OP.subtract)
        nc.vector.tensor_scalar_add(var[:], var[:], 1e-5)
        rsq_i = rsqrt_dve(var, tag)
        rstd = rsq_i[:].bitcast(F32)
        mbneg = st_p.tile([128, NT], F32, tag=f"mb_{tag}")
        nc.vector.scalar_tensor_tensor(mbneg[:], mean[:], -1.0, rstd,
                                       op0=OP.mult, op1=OP.mult)
        return rsq_i, mbneg

    def ln_finish(stats6, tag):
        """stats6 [128, NT, 6] from bn_stats (2 groups of count/mean/M2).
        -> (rstd-int tile (bitcast f32), mbneg f32 [128,NT])."""
        mv2 = st_p.tile([128, NT, 2], F32, tag=f"mv_{tag}")
        for n in range(NT):
            nc.vector.bn_aggr(mv2[:, n, :], stats6[:, n, :])
        mv = mv2[:, :, 0:1].rearrange("p n o -> p (n o)")
        var = st_p.tile([128, NT], F32, tag=f"var_{tag}")
        nc.vector.tensor_scalar(var[:],
                                mv2[:, :, 1:2].rearrange("p n o -> p (n o)"),
                                1.0, 1e-5, op0=OP.mult, op1=OP.add)
        rsq_i = rsqrt_dve(var, tag)
        rstd = rsq_i[:].bitcast(F32)
        mbneg = st_p.tile([128, NT], F32, tag=f"mb_{tag}")
        nc.vector.scalar_tensor_tensor(mbneg[:], mv, -1.0, rstd,
                                       op0=OP.mult, op1=OP.mult)
        return rsq_i, mbneg

    def ln_bounce(xa_t, rsq_i, mbneg, which, b):
        """normalize (ACT) -> bounce -> transpose -> delta. Returns
        (xnB list, d list)."""
        rstd = rsq_i[:].bitcast(F32)
        xn = xn_p.tile([128, NT, C], BF16, tag=f"xn_{which}")
        for n in range(NT):
            nc.scalar.activation(xn[:, n, :], xa_t[:, n, :], AF.Identity,
                                 bias=mbneg[:, n:n + 1],
                                 scale=rstd[:, n:n + 1])
        xnd = dramp.tile([T + 32, C], BF16, tag=f"xnd_{which}")
        nc.sync.dma_start(xnd[0:32, :], zrow[:])
        nc.sync.dma_start(xnd[32:T + 32].rearrange("(n p) c -> p n c", p=128),
                          xn[:])
        xnB, dB = [], []
        for cc in range(CC):
            t_ = xb_p.tile([128, T + 32], BF16, tag=f"xnB_{which}_{cc}")
            nc.sync.dma_start_transpose(t_[:],
                                        xnd[:, cc * 128:(cc + 1) * 128])
            xnB.append(t_)
            d = xb_p.tile([128, T], BF16, tag=f"d_{which}_{cc}")
            nc.gpsimd.tensor_tensor(d[:], t_[:, 32:T + 32], t_[:, 31:T + 31],
                                    op=OP.subtract)
            dB.append(d)
        return xnB, dB

    def mix8(xnB, dB, coefm1, tag, b, dt=FP8):
        """mix tile [128, CC, T]: out = xn + (coef-1)*d."""
        m = mx_p.tile([128, CC, T], dt, tag=f"mx_{tag}")
        for cc in range(CC):
            nc.vector.scalar_tensor_tensor(m[:, cc, :], dB[cc][:],
                                           coefm1[cc][:],
                                           xnB[cc][:, 32:T + 32],
                                           op0=OP.mult, op1=OP.add)
        return m

    def mix8_half(m, xnB, dB, coefm1, th):
        """fill th-half of a mix tile: cols th*512..(th+1)*512."""
        sl = slice(th * 512, (th + 1) * 512)
        sl32 = slice(32 + th * 512, 32 + (th + 1) * 512)
        for cc in range(CC):
            nc.vector.scalar_tensor_tensor(m[:, cc, sl], dB[cc][:, sl],
                                           coefm1[cc][:],
                                           xnB[cc][:, sl32],
                                           op0=OP.mult, op1=OP.add)

    def bf_group(out_ps, wb_, mb_, hh, th):
        for ci in range(CC):
            nc.tensor.matmul(
                out_ps[:],
                wb_[:, ci, hh * 128:(hh + 1) * 128],
                mb_[:, ci, th * 512:(th + 1) * 512],
                start=(ci == 0), stop=(ci == CC - 1))

    def dr_group(out_ps, w8, m8, hh, th, nsteps=CC // 2):
        """Accumulate DR matmuls: out += w8[:,2s:2s+2, hh*128:+128].T @
        m8[:,2s:2s+2, th*512:+512] over s."""
        for s in range(nsteps):
            nc.tensor.matmul(
                out_ps[:],
                w8[:, 2 * s:2 * s + 2, hh * 128:(hh + 1) * 128],
                m8[:, 2 * s:2 * s + 2, th * 512:(th + 1) * 512],
                start=(s == 0), stop=(s == nsteps - 1),
                perf_mode=PM.DoubleRow)

    # ================= per-batch stages (chunked) =================

    def chunks_A(b):
        def a00():
            if b in state and "xa" in state[b]:
                return
            xb = x_d[b].rearrange("(n p) c -> p n c", p=128)
            xa = xa_p.tile([128, NT, C], F32, tag="xa", name=f"xa_{b}")
            nc.sync.dma_start(xa[:], xb)
            state[b] = dict(xa=xa)

        def a0():
            xa = state[b]["xa"]
            stats6 = st_p.tile([128, NT, 6], F32, tag="stats1")
            for n in range(NT):
                nc.vector.bn_stats(stats6[:, n, :], xa[:, n, :])
            state[b]["stats6"] = stats6

        def a1a():
            s = state[b]
            rsq_i, mbneg = ln_finish(s["stats6"], "1")
            s["rsq_i"], s["mbneg"] = rsq_i, mbneg
            xn = xn_p.tile([128, NT, C], BF16, tag="xn_1")
            s["xn"] = xn
            rstd = rsq_i[:].bitcast(F32)
            for n in range(NT // 2):
                nc.scalar.activation(xn[:, n, :], s["xa"][:, n, :],
                                     AF.Identity, bias=mbneg[:, n:n + 1],
                                     scale=rstd[:, n:n + 1])

        def a1b():
            s = state[b]
            xn = s["xn"]
            rstd = s["rsq_i"][:].bitcast(F32)
            mbneg = s["mbneg"]
            for n in range(NT // 2, NT):
                nc.scalar.activation(xn[:, n, :], s["xa"][:, n, :],
                                     AF.Identity, bias=mbneg[:, n:n + 1],
                                     scale=rstd[:, n:n + 1])
            xnd = dramp.tile([T + 32, C], BF16, tag="xnd_1")
            nc.sync.dma_start(xnd[0:32, :], zrow[:])
            nc.sync.dma_start(
                xnd[32:T + 32].rearrange("(n p) c -> p n c", p=128), xn[:])
            s["xnd"] = xnd

        def a2():
            s = state[b]
            xnd = s["xnd"]
            xnB, dB = [], []
            for cc in range(CC):
                t_ = xb_p.tile([128, T + 32], BF16, tag=f"xnB_{cc}")
                nc.sync.dma_start_transpose(t_[:],
                                            xnd[:, cc * 128:(cc + 1) * 128])
                xnB.append(t_)
                d = xb_p.tile([128, T], BF16, tag=f"d_{cc}")
                nc.vector.tensor_tensor(d[:], t_[:, 32:T + 32],
                                        t_[:, 31:T + 31], op=OP.subtract)
                dB.append(d)
            xn8 = mx_p.tile([128, CC, T + 32], FP8, tag="xn8")
            for cc in range(CC):
                nc.scalar.activation(xn8[:, cc, :], xnB[cc][:], AF.Identity)
            s["xn8"] = xn8
            s["xnB"], s["dB"] = xnB, dB

        def a3():
            s = state[b]
            xnB, dB = s["xnB"], s["dB"]
            s["xk8"] = mix8(xnB, dB, tmk_c, "k", b, dt=BF16)
            s["xv8"] = mix8(xnB, dB, tmv_c, "v", b, dt=BF16)

        return [a00, a0, a1a, a1b, a2, a3]

    def chunks_B(b):
        def mk_hh(hh):
            def f():
                s = state[b]
                xk8, xv8, xn8 = s["xk8"], s["xv8"], s["xn8"]
                if "srw8" not in s:
                    s["srw8"] = srw_p.tile([128, CC, T], FP8, tag="srw8",
                                           name=f"srw_{b}")
                srw8 = s["srw8"]
                e = wk_p.tile([128, T], BF16, tag="e")
                th_t = wk_p.tile([128, T], BF16, tag="tht")
                vsb = wk_p.tile([128, T], BF16, tag="vsb")
                ev = wk_p.tile([128, T], BF16, tag="ev")
                for th in range(2):
                    sl = slice(th * 512, (th + 1) * 512)
                    k_ps = ps.tile([128, 512], F32, tag="ps",
                                   name=f"kps{b}_{hh}{th}")
                    bf_group(k_ps, wkb, xk8, hh, th)
                    r_ps = ps.tile([128, 512], F32, tag="ps",
                                   name=f"rps{b}_{hh}{th}")
                    for s_ in range(CC // 2):
                        nc.tensor.matmul(
                            r_ps[:],
                            wr8a[:, 2 * s_:2 * s_ + 2,
                                 hh * 128:(hh + 1) * 128],
                            xn8[:, 2 * s_:2 * s_ + 2,
                                32 + th * 512:32 + (th + 1) * 512],
                            start=(s_ == 0), stop=False,
                            perf_mode=PM.DoubleRow)
                    for s_ in range(CC // 2):
                        nc.tensor.matmul(
                            r_ps[:],
                            wr8b[:, 2 * s_:2 * s_ + 2,
                                 hh * 128:(hh + 1) * 128],
                            xn8[:, 2 * s_:2 * s_ + 2,
                                31 + th * 512:31 + (th + 1) * 512],
                            start=False, stop=(s_ == CC // 2 - 1),
                            perf_mode=PM.DoubleRow)
                    v_ps = ps.tile([128, 512], F32, tag="ps",
                                   name=f"vps{b}_{hh}{th}")
                    bf_group(v_ps, wvb, xv8, hh, th)
                    nc.scalar.activation(e[:, sl], k_ps[:], AF.Exp)
                    nc.scalar.activation(th_t[:, sl], r_ps[:], AF.Tanh,
                                         scale=0.5 / WS)
                    nc.scalar.activation(vsb[:, sl], v_ps[:], AF.Identity)
                nc.vector.tensor_tensor(ev[:], e[:], vsb[:], op=OP.mult)
                if False:
                    pass
                Pb = sc_p.tile([128, T + 1], F32, tag="Pb")
                Qb = sc_p.tile([128, T + 1], F32, tag="Qb")
                nc.vector.memset(Pb[:, 0:1], 0.0)
                nc.vector.memset(Qb[:, 0:1], 0.0)
                db = delta_c[hh][:].to_broadcast((128, T))
                nc.vector.tensor_tensor_scan(Pb[:, 1:T + 1], db, ev[:],
                                             0.0, op0=OP.mult, op1=OP.add)
                nc.vector.tensor_tensor_scan(Qb[:, 1:T + 1], db, e[:],
                                             0.0, op0=OP.mult, op1=OP.add)
                Nt = wt_p.tile([128, T], BF16, tag="Nt")
                Dt = wt_p.tile([128, T], F32, tag="Dt")
                nc.vector.scalar_tensor_tensor(Nt[:], ev[:], eu_c[hh][:],
                                               Pb[:, 0:T], op0=OP.mult,
                                               op1=OP.add)
                nc.vector.scalar_tensor_tensor(Dt[:], e[:], eu_c[hh][:],
                                               Qb[:, 0:T], op0=OP.mult,
                                               op1=OP.add)
                nc.vector.reciprocal_approx_fast(Dt[:], Dt[:])
                yp = wt_p.tile([128, T], BF16, tag="yp")
                nc.vector.tensor_tensor(yp[:], Nt[:], Dt[:], op=OP.mult)
                yth = wt_p.tile([128, T], BF16, tag="yth")
                nc.vector.tensor_tensor(yth[:], yp[:], th_t[:], op=OP.mult)
                nc.vector.tensor_tensor(srw8[:, hh, :], yp[:], yth[:],
                                        op=OP.add)
            return f

        def wo():
            s = state[b]
            xa, srw8 = s["xa"], s["srw8"]
            stats6 = st_p.tile([128, NT, 6], F32, tag="stats2")
            s["stats62"] = stats6
            for n in range(NT):
                p_ = ps.tile([128, 512], F32, tag="ps", name=f"wops{b}_{n}")
                for s_ in range(CC // 2):
                    nc.tensor.matmul(
                        p_[:],
                        srw8[:, 2 * s_:2 * s_ + 2, n * 128:(n + 1) * 128],
                        wo8[:, 2 * s_:2 * s_ + 2, :],
                        start=(s_ == 0), stop=(s_ == CC // 2 - 1),
                        perf_mode=PM.DoubleRow)
                nc.vector.scalar_tensor_tensor(xa[:, n, :], p_[:], 1.0 / WS,
                                               xa[:, n, :], op0=OP.mult,
                                               op1=OP.add)
                nc.vector.bn_stats(stats6[:, n, :], xa[:, n, :])

        return [mk_hh(0), mk_hh(1), mk_hh(2), mk_hh(3), wo]

    def chunks_C(b):
        def c0a():
            s = state[b]
            xa = s["xa"]
            rsq_i, mbneg = ln_finish(s["stats62"], "2")
            rstd = rsq_i[:].bitcast(F32)
            s["rsq2"], s["mb2"] = rsq_i, mbneg
            xn = xn_p.tile([128, NT, C], BF16, tag="xn_2")
            s["xn2"] = xn
            xnd = dramp.tile([T + 32, C], BF16, tag="xnd_2")
            s["xnd2"] = xnd
            for n in range(NT // 2):
                nc.scalar.activation(xn[:, n, :], xa[:, n, :], AF.Identity,
                                     bias=mbneg[:, n:n + 1],
                                     scale=rstd[:, n:n + 1])
            nc.sync.dma_start(xnd[0:32, :], zrow[:])
            nc.sync.dma_start(
                xnd[32:32 + 512].rearrange("(n p) c -> p n c", p=128),
                xn[:, 0:NT // 2, :])

        def c0b():
            s = state[b]
            xa, xn, xnd = s["xa"], s["xn2"], s["xnd2"]
            rstd = s["rsq2"][:].bitcast(F32)
            mbneg = s["mb2"]
            for n in range(NT // 2, NT):
                nc.scalar.activation(xn[:, n, :], xa[:, n, :], AF.Identity,
                                     bias=mbneg[:, n:n + 1],
                                     scale=rstd[:, n:n + 1])
            nc.sync.dma_start(
                xnd[32 + 512:T + 32].rearrange("(n p) c -> p n c", p=128),
                xn[:, NT // 2:NT, :])

        def c1a():
            s = state[b]
            xnd = s["xnd2"]
            xnB, dB = [], []
            for cc in range(CC):
                t_ = xb_p.tile([128, T + 32], BF16, tag=f"xnB_{cc}")
                nc.sync.dma_start_transpose(
                    t_[:, 0:544], xnd[0:544, cc * 128:(cc + 1) * 128])
                xnB.append(t_)
                d = xb_p.tile([128, T], BF16, tag=f"d_{cc}")
                nc.vector.tensor_tensor(d[:, 0:512], t_[:, 32:544],
                                        t_[:, 31:543], op=OP.subtract)
                dB.append(d)
            s["xnB2"], s["dB2"] = xnB, dB
            xn28 = mx_p.tile([128, CC, T + 32], FP8, tag="xn28")
            s["xn28"] = xn28
            for cc in range(CC):
                nc.scalar.activation(xn28[:, cc, 0:544], xnB[cc][:, 0:544],
                                     AF.Identity)
            xk28 = mx_p.tile([128, CC, T], FP8, tag="mx_k2")
            s["xk28"] = xk28
            mix8_half(xk28, xnB, dB, cmk_c, 0)

        def c1b():
            s = state[b]
            xnd = s["xnd2"]
            xnB, dB = s["xnB2"], s["dB2"]
            for cc in range(CC):
                t_ = xnB[cc]
                nc.sync.dma_start_transpose(
                    t_[:, 544:T + 32],
                    xnd[544:T + 32, cc * 128:(cc + 1) * 128])
                nc.vector.tensor_tensor(dB[cc][:, 512:1024],
                                        t_[:, 544:T + 32],
                                        t_[:, 543:T + 31], op=OP.subtract)
            xn28 = s["xn28"]
            for cc in range(CC):
                nc.scalar.activation(xn28[:, cc, 544:T + 32],
                                     xnB[cc][:, 544:T + 32], AF.Identity)
            mix8_half(s["xk28"], xnB, dB, cmk_c, 1)

        return [c0a, c0b, c1a, c1b]

    def chunks_D(b):
        def mk_k(th, part):
            def f():
                s = state[b]
                xk28 = s["xk28"]
                if f"kk8_{th}" not in s:
                    s[f"kk8_{th}"] = kk_p.tile([128, HC, 512], FP8,
                                               tag="kk8",
                                               name=f"kk8_{b}_{th}")
                kk8 = s[f"kk8_{th}"]
                for hh in range(part * 4, part * 4 + 4):
                    p_ = ps.tile([128, 512], F32, tag="ps",
                                 name=f"ck{b}_{th}{hh}")
                    dr_group(p_, cwk8, xk28, hh, th)
                    kkb = kk_p.tile([128, 512], BF16, tag="kkb")
                    nc.scalar.activation(kkb[:], p_[:], AF.Relu,
                                         scale=1.0 / WS)
                    nc.scalar.activation(kk8[:, hh, :], kkb[:], AF.Square,
                                         scale=1.0 / 8.0)
            return f

        def mk_rv(th):
            def f():
                s = state[b]
                xa, xn28 = s["xa"], s["xn28"]
                kk8 = s[f"kk8_{th}"]
                yb = y_d[b].rearrange("(n p) c -> p n c", p=128)
                for nn in range(4):
                    n = th * 4 + nn
                    rp = ps.tile([128, 512], F32, tag="ps", name=f"cr{b}_{n}")
                    for s_ in range(CC // 2):
                        nc.tensor.matmul(
                            rp[:],
                            xn28[:, 2 * s_:2 * s_ + 2,
                                 32 + n * 128:32 + (n + 1) * 128],
                            cwr8a[:, 2 * s_:2 * s_ + 2, :],
                            start=(s_ == 0), stop=False,
                            perf_mode=PM.DoubleRow)
                    for s_ in range(CC // 2):
                        nc.tensor.matmul(
                            rp[:],
                            xn28[:, 2 * s_:2 * s_ + 2,
                                 31 + n * 128:31 + (n + 1) * 128],
                            cwr8b[:, 2 * s_:2 * s_ + 2, :],
                            start=False, stop=(s_ == CC // 2 - 1),
                            perf_mode=PM.DoubleRow)
                    sig2 = out_p.tile([128, 512], BF16, tag="sig2")
                    nc.scalar.activation(sig2[:], rp[:], AF.Sigmoid,
                                         scale=1.0 / WS)
                    kvp = ps.tile([128, 512], F32, tag="ps", name=f"cv{b}_{n}")
                    for s_ in range(HC // 2):
                        nc.tensor.matmul(
                            kvp[:],
                            kk8[:, 2 * s_:2 * s_ + 2,
                                nn * 128:(nn + 1) * 128],
                            cwv8[:, 2 * s_:2 * s_ + 2, :],
                            start=(s_ == 0), stop=(s_ == HC // 2 - 1),
                            perf_mode=PM.DoubleRow)
                    t2 = out_p.tile([128, 512], BF16, tag="t2")
                    nc.vector.tensor_tensor(t2[:], kvp[:], sig2[:],
                                            op=OP.mult)
                    yo = out_p.tile([128, 512], F32, tag="yo")
                    nc.gpsimd.tensor_tensor(yo[:], t2[:], xa[:, n, :],
                                            op=OP.add)
                    nc.sync.dma_start(yb[:, n, :], yo[:])
            return f

        return [mk_k(0, 0), mk_k(0, 1), mk_k(0, 2), mk_k(0, 3), mk_rv(0),
                mk_k(1, 0), mk_k(1, 1), mk_k(1, 2), mk_k(1, 3), mk_rv(1)]

    # software-pipelined, chunk-interleaved emission:
    #   cycle b: round-robin over B(b), D(b-1), A(b+1)[a0..a2];
    #   then A(b+1).a3 (mixes), then C(b).
    from itertools import zip_longest
    for ch in chunks_A(0):
        ch()
    prev_D = None
    for b in range(bl):
        Bc = chunks_B(b)
        Dc = prev_D or []
        Ac_all = chunks_A(b + 1) if b + 1 < bl else []
        if Ac_all:
            Ac_all[0]()  # xa load for b+1, two cycles ahead of its D
        Ac, Amix = Ac_all[1:5], Ac_all[5:]
        for group in zip_longest(Bc, Dc, Ac):
            for ch in group:
                if ch is not None:
                    ch()
        for ch in Amix:
            ch()
        Cc = chunks_C(b)
        if b == bl - 1:
            Dl = chunks_D(b)
            # c0a c0b c1a k0a k0b c1b k0c k0d rv0 k1...
            for ch in [Cc[0], Cc[1], Cc[2], Dl[0], Dl[1], Cc[3], Dl[2],
                       Dl[3], Dl[4], Dl[5], Dl[6], Dl[7], Dl[8], Dl[9]]:
                ch()
            prev_D = []
        else:
            for ch in Cc:
                ch()
            prev_D = chunks_D(b)
    for ch in prev_D:
        ch()


def build_program(bl=BL):
    nc = bacc.Bacc("TRN2", target_bir_lowering=False, debug=False,
                   num_devices=NCORES)
    io = {}
    io["x"] = nc.dram_tensor("x", [bl, T, C], F32, kind="ExternalInput")
    io["y"] = nc.dram_tensor("y", [bl, T, C], F32, kind="ExternalOutput")
    for nm, shp in [("wr8a", [128, CC, C]), ("wr8b", [128, CC, C]),
                    ("wo8", [128, CC, C]),
                    ("cwk8", [128, CC, H]), ("cwr8a", [128, CC, C]),
                    ("cwr8b", [128, CC, C]), ("cwv8", [128, HC, C])]:
        io[nm] = nc.dram_tensor(nm, shp, FP8, kind="ExternalInput")
    for nm in ["wkb", "wvb"]:
        io[nm] = nc.dram_tensor(nm, [128, CC, C], BF16, kind="ExternalInput")
    for nm in ["delta", "expu", "tmk", "tmv", "tmr", "cmk", "cmr"]:
        io[nm] = nc.dram_tensor(nm, [C], F32, kind="ExternalInput")
    import os
    if os.environ.get("KDBG"):
        io["dbg_xnB0"] = nc.dram_tensor("dbg_xnB0", [128, T + 32], BF16,
                                        kind="ExternalOutput")
        io["dbg_xk8"] = nc.dram_tensor("dbg_xk8", [128, CC, T], BF16,
                                       kind="ExternalOutput")
        for nm in ["dbg_e", "dbg_tht", "dbg_ev", "dbg_Nt", "dbg_yp"]:
            io[nm] = nc.dram_tensor(nm, [128, T], BF16 if nm != "dbg_ev" else BF16,
                                    kind="ExternalOutput")
        io["dbg_Pb"] = nc.dram_tensor("dbg_Pb", [128, T + 1], F32,
                                      kind="ExternalOutput")
        io["dbg_x1"] = nc.dram_tensor("dbg_x1", [128, NT, C], F32,
                                      kind="ExternalOutput")
        io["dbg_srw8"] = nc.dram_tensor("dbg_srw8", [128, CC, T], FP8,
                                        kind="ExternalOutput")

    with tile.TileContext(nc) as tc:
        with ExitStack() as ctx:
            _emit(nc, tc, ctx, io, bl)
    nc.compile()
    return nc


def host_params(inputs):
    f32 = np.float32
    g1 = np.asarray(inputs["ln1_g"], f32)
    b1 = np.asarray(inputs["ln1_b"], f32)
    g2 = np.asarray(inputs["ln2_g"], f32)
    b2 = np.asarray(inputs["ln2_b"], f32)
    assert np.allclose(b1, 0.0) and np.allclose(b2, 0.0), \
        "nonzero LN bias not supported"
    Wk = np.asarray(inputs["Wk"], f32)
    Wv = np.asarray(inputs["Wv"], f32)
    Wr = np.asarray(inputs["Wr"], f32)
    Wo = np.asarray(inputs["Wo"], f32)
    cWk = np.asarray(inputs["cWk"], f32)
    cWr = np.asarray(inputs["cWr"], f32)
    cWv = np.asarray(inputs["cWv"], f32)

    fp8 = ml_dtypes.float8_e4m3

    def dr8(WT, scale):
        # WT [K, M] -> [128, K//128, M], k = j*128 + p
        K = WT.shape[0]
        return np.ascontiguousarray(
            (WT * scale).reshape(K // 128, 128, -1).transpose(1, 0, 2)
        ).astype(fp8)

    bfd = ml_dtypes.bfloat16

    def drb(WT):
        K = WT.shape[0]
        return np.ascontiguousarray(
            WT.reshape(K // 128, 128, -1).transpose(1, 0, 2)).astype(bfd)

    p = {
        "wkb": drb(Wk.T * g1[:, None]),
        "wvb": drb(Wv.T * g1[:, None]),
        "wr8a": dr8(Wr.T * (g1 * np.asarray(inputs["tm_r"], f32))[:, None],
                    WS),
        "wr8b": dr8(Wr.T * (g1 * (1.0 - np.asarray(inputs["tm_r"],
                                                   f32)))[:, None], WS),
        "wo8": dr8(Wo.T, WS * 0.5),
        "cwk8": dr8(cWk.T * g2[:, None], WS),
        "cwr8a": dr8(cWr.T * (g2 * np.asarray(inputs["cm_r"], f32))[:, None],
                     WS),
        "cwr8b": dr8(cWr.T * (g2 * (1.0 - np.asarray(inputs["cm_r"],
                                                     f32)))[:, None], WS),
        "cwv8": dr8(cWv.T, WS),
        "delta": np.exp(-np.exp(np.asarray(inputs["time_decay"], f32))),
        "expu": np.exp(np.asarray(inputs["time_first"], f32)),
        "tmk": np.asarray(inputs["tm_k"], f32),
        "tmv": np.asarray(inputs["tm_v"], f32),
        "tmr": np.asarray(inputs["tm_r"], f32),
        "cmk": np.asarray(inputs["cm_k"], f32),
        "cmr": np.asarray(inputs["cm_r"], f32),
    }
    return p


WS = 64.0
_CACHE = {}


def kernel(**inputs):
    from concourse.bass_utils import run_bass_kernel_spmd

    if "nc" not in _CACHE:
        _CACHE["nc"] = build_program(BL)
    nc = _CACHE["nc"]

    p = host_params(inputs)
    x = np.asarray(inputs["x"], np.float32)
    in_maps = []
    for c in range(NCORES):
        m = dict(p)
        m["x"] = np.ascontiguousarray(x[c * BL:(c + 1) * BL])
        in_maps.append(m)
    res = run_bass_kernel_spmd(nc, in_maps, list(range(NCORES)))
    out = np.concatenate([res.results[c]["y"] for c in range(NCORES)], axis=0)
    return out.astype(np.float32)
